# revision 39
# baseline (speedup 1.0000x reference)
"""Trainium2 Bass kernel for nn_Attention_79224966742132.

Dense transformer attention block: QKV projection + axial RoPE + SDPA +
output projection, for x (2, 2048, 1152), 16 heads of dim 72.

Sharding (8 cores): data-parallel over batch (2) x tensor-parallel over
head groups (4 heads/core). Each core computes QKV for its 4 heads from
the full x[b], applies RoPE, runs attention, and produces a partial
output projection (row-parallel Wproj); the host sums the 4 partials per
batch element. The projection bias rides as an extra contraction row on
the g==0 core of each batch.

v3 design notes (against the TimelineSim cost model):
- All phase-1 matmuls in fp16 (1 cycle/row at any moving size); x, Wqkv,
  Wv are quantized to fp16 on the host (~1e-3 rel err, gate is 2e-2).
- QK projection packed into 5 stationary blocks of <=128 columns
  (4x128 + 64) instead of 6x96: pass-dims fill the block remainders.
- Attention-value matmul restructured: exp-weights tile [128kt, 128qt]
  is the STATIONARY operand, v [128, 73] fp16 the moving one -> 73
  cycles per k-tile instead of 512 (output lands as [qtok, hd]; a cheap
  PE transpose brings it back to [hd, qtok] for the projection).
- Softmax denominator = ones column appended to v; reciprocal + scale on
  DVE in the [qtok, hd] layout (per-partition scalar, no broadcasts).
- Projection bias folded into the h3 projection matmul as a 73rd
  contraction row against a ones row in o16[3].
- The exp on ACT (133us) is the phase-2 near-critical path; V for token
  halves 2-3, all projection matmuls, and each chunk's AV/normalize/
  transpose tail are deferred into a filler queue drained between score
  matmuls so PE never idles while ACT catches up.
- Half-0 scores+exps for chunks (h0,jq0)/(h1,jq0) are emitted inside
  phase 1 (hooked between qn3's blocks) so ACT starts ~20us early.
- HWDGE descriptor-gen is a single serialized device (~625ns/DMA) and
  DMA transfers serialize on one modeled DMA_ENGINES device: DMA count
  is minimized (wv merged into the wqk tensor, wp packed into one
  tile); latency-critical repacks go HWDGE, the rest SWDGE (Pool).
- PSUM accumulation groups are zeroed per BANK by matmul start=True, so
  the four AV sub-tile groups sharing one bank are memset once and
  accumulated with start=False.
- GPSIMD cannot touch PSUM and engine APs need 32-aligned partition
  offsets (hardware rules the cost model does not check).
"""
import math
import os
import sys
from collections import deque

# The device path needs the axon/neuron jax platform; if a harness pinned
# JAX_PLATFORMS=cpu (common for running jax references) and jax is not yet
# imported, restore platform auto-detection.
if "jax" not in sys.modules:
    _jp = os.environ.get("JAX_PLATFORMS")
    if _jp and "axon" not in _jp and "neuron" not in _jp:
        del os.environ["JAX_PLATFORMS"]

import numpy as np

import bass_rust
import concourse.bass as bass
import concourse.mybir as mybir
import concourse.tile as tile
from concourse.bass_utils import run_bass_kernel_spmd
from concourse.masks import make_identity

F32 = mybir.dt.float32
F16 = mybir.dt.float16
AF = mybir.ActivationFunctionType
ALU = mybir.AluOpType

B = 2
N = 2048          # tokens = T*H*W = 8*16*16
C = 1152
NH = 16
HD = 72
HPG = 4           # heads per core
NCORES = 8
GT, GH, GW = 8, 16, 16
SCALE = 1.0 / math.sqrt(HD)

NQ = 4            # q-chunks (512 tokens each) and qt-subtiles per chunk
QS = N // NQ      # 512
KTILES = N // 128  # 16
CK = C // 128      # 9 contraction chunks
HS = N // 2        # RoPE-output/repack half granularity


def _axis_freqs(n: int) -> np.ndarray:
    base = np.linspace(1.0, 128.0, 8, dtype=np.float64) * np.pi   # MAX_FREQ/2
    pos = np.linspace(-1.0, 1.0, n, dtype=np.float64)
    return pos[:, None] * base[None, :]                            # (n, 8)


def _cos_sin_96():
    """cos/sin of the 24 pair frequencies per token, tiled x4 -> (96, N)."""
    f = np.zeros((GT, GH, GW, 24), dtype=np.float64)
    f[..., 0:8] = _axis_freqs(GT)[:, None, None, :]
    f[..., 8:16] = _axis_freqs(GH)[None, :, None, :]
    f[..., 16:24] = _axis_freqs(GW)[None, None, :, :]
    f = f.reshape(N, 24)
    cos24 = np.ascontiguousarray(np.cos(f).astype(np.float32).T)   # (24, N)
    sin24 = np.ascontiguousarray(np.sin(f).astype(np.float32).T)
    return np.tile(cos24, (4, 1)), np.tile(sin24, (4, 1))          # (96, N)


def build_nc() -> bass.Bass:
    nc = bass.Bass()
    xT = nc.dram_tensor("xT", [C, N], F16, kind="ExternalInput")
    wqk = nc.dram_tensor("wqk", [C, 576 + HPG * HD], F16, kind="ExternalInput")
    wp = nc.dram_tensor("wp", [HD + 1, HPG * C], F16, kind="ExternalInput")
    cosd = nc.dram_tensor("cosd", [96, N], F16, kind="ExternalInput")
    sind = nc.dram_tensor("sind", [96, N], F16, kind="ExternalInput")
    outT = nc.dram_tensor("outT", [C, N], F32, kind="ExternalOutput")

    with tile.TileContext(nc) as tc:
        with tc.tile_pool(name="persist", bufs=1) as pp:
            x16 = [pp.tile([128, N], F16, name=f"x16_{k}") for k in range(CK)]
            qt16 = pp.tile([HD, HPG * N], F16, name="qt16")
            kt16 = pp.tile([HD, HPG * N], F16, name="kt16")
            v16 = [pp.tile([128, HPG, HD + 1], F16, name=f"v16_{i}")
                   for i in range(KTILES)]
            o16 = [pp.tile([HD + (1 if h == 3 else 0), N], F16, name=f"o16_{h}")
                   for h in range(HPG)]
            wqk_t = [pp.tile([128, 576 + HPG * HD], F16, name=f"wqk{k}")
                     for k in range(CK)]
            wp4 = pp.tile([HD + 1, HPG, C], F16, name="wp4")
            cos_t = pp.tile([96, N], F16, name="cos_t")
            sin_t = pp.tile([96, N], F16, name="sin_t")
            ident = pp.tile([128, 128], F16, name="ident")

            ones_row = pp.tile([1, N], F16, name="ones_row")
            make_identity(nc, ident[:])
            for i in range(KTILES):
                nc.vector.memset(v16[i][:, :, HD], 1.0)
            nc.vector.memset(ones_row[:], 1.0)
            # engine writes need 32-aligned partition offsets; DMA does not
            nc.sync.dma_start(o16[3][HD:HD + 1, :], ones_row[:])

            # bulk loads alternate between the two HWDGE issuers (SP + ACT,
            # 16 queues each) so transfers run in parallel and neither
            # sequencer serializes the load phase
            _eng = [nc.sync, nc.scalar]
            _ei = [0]

            def dma(out, in_):
                _eng[_ei[0] & 1].dma_start(out, in_)
                _ei[0] += 1

            for k in range(CK):
                dma(wqk_t[k][:], wqk[k * 128:(k + 1) * 128, :])
                dma(x16[k][:, 0:HS], xT[k * 128:(k + 1) * 128, 0:HS])
            dma(cos_t[:], cosd[:, :])
            dma(sin_t[:], sind[:, :])
            for k in range(CK):
                dma(x16[k][:, HS:N], xT[k * 128:(k + 1) * 128, HS:N])
            dma(wp4[:], wp[:].rearrange("p (h c) -> p h c", h=HPG))

            # ---------------- emit helpers ----------------

            def emit_qkrope(ps_pool, sb_pool, qn, halves, hook=None):
                """5-block QK matmuls + RoPE for one token quarter, 4 heads.

                Column blocks (stationary, host-packed):
                  B0 = Qe(96) + Qp[0:32]     B1 = Qo(96) + Qp[32:64]
                  B2 = Ke(96) + Qp[64:96]    B3 = Ko(96) + Kp[0:32]
                  B4 = Kp[32:96]
                where e/o/p = rotary-even/odd/pass dims, head-major.
                RoPE for Q is emitted right after B1 (and K after B3) so the
                DVE chain starts early and single-buffered PSUM blocks never
                stall the next quarter.
                """
                ts0 = qn * QS
                hn, sub = divmod(qn, 2)
                sl = slice(sub * QS, (sub + 1) * QS)
                erq, orq, prq, erk, ork, prk = halves[hn]
                cosq = cos_t[:, ts0:ts0 + QS]
                sinq = sin_t[:, ts0:ts0 + QS]

                def mm_block(m):
                    w = 64 if m == 4 else 128
                    blk = ps_pool.tile([w, QS], F32, tag=f"qk{m}", bufs=1,
                                       name=f"qk{qn}_{m}")
                    for k in range(CK):
                        nc.tensor.matmul(
                            blk[:],
                            wqk_t[k][:, 128 * m:128 * m + w],
                            x16[k][:, ts0:ts0 + QS],
                            start=(k == 0), stop=(k == CK - 1),
                        )
                    return blk

                def rope(e_blk, o_blk, er, orr):
                    t1 = sb_pool.tile([96, QS], F16, tag="rtA", bufs=1,
                                      name=f"t1_{qn}")
                    t2 = sb_pool.tile([96, QS], F16, tag="rtB", bufs=1,
                                      name=f"t2_{qn}")
                    nc.vector.tensor_tensor(t1[:], e_blk[0:96, :], cosq, ALU.mult)
                    nc.vector.tensor_tensor(t2[:], o_blk[0:96, :], sinq, ALU.mult)
                    nc.vector.tensor_tensor(er[:, sl], t1[:], t2[:], ALU.subtract)
                    t3 = sb_pool.tile([96, QS], F16, tag="rtA", bufs=1,
                                      name=f"t3_{qn}")
                    t4 = sb_pool.tile([96, QS], F16, tag="rtB", bufs=1,
                                      name=f"t4_{qn}")
                    nc.vector.tensor_tensor(t3[:], o_blk[0:96, :], cosq, ALU.mult)
                    nc.vector.tensor_tensor(t4[:], e_blk[0:96, :], sinq, ALU.mult)
                    nc.vector.tensor_tensor(orr[:, sl], t3[:], t4[:], ALU.add)

                if qn == 3:
                    # K first: the half-1 kt repack gates phase 2
                    B2 = mm_block(2)
                    if hook: hook()
                    B3 = mm_block(3)
                    rope(B2, B3, erk, ork)
                    if hook: hook()
                    B0 = mm_block(0)
                    if hook: hook()
                    B1 = mm_block(1)
                    rope(B0, B1, erq, orq)
                    if hook: hook()
                else:
                    B0 = mm_block(0)
                    B1 = mm_block(1)
                    rope(B0, B1, erq, orq)
                    B2 = mm_block(2)
                    B3 = mm_block(3)
                    rope(B2, B3, erk, ork)
                B4 = mm_block(4)
                # pass dims: Qp spread over B0/B1/B2 remainders, Kp over B3/B4.
                # The last quarter's copies go to the (idle) ACT engine so the
                # PSUM banks free up fast for phase 2.
                nc.scalar.copy(prq[0:32, sl], B0[96:128, :])
                nc.scalar.copy(prq[32:64, sl], B1[96:128, :])
                nc.scalar.copy(prq[64:96, sl], B2[96:128, :])
                nc.scalar.copy(prk[0:32, sl], B3[96:128, :])
                nc.scalar.copy(prk[32:64, sl], B4[0:32, :])
                nc.scalar.copy(prk[64:96, sl], B4[32:64, :])

            def emit_repack(hn, halves, part="both"):
                """DMA the rotated halves into per-head [72, N] q/k tiles.

                Per-head dim order: [0:24] even-rotated, [24:48] odd-rotated,
                [48:72] pass -- same permutation for q and k, so scores are
                unchanged. Issues alternate between the two HWDGE engines.
                """
                erq, orq, prq, erk, ork, prk = halves[hn]
                hs0 = hn * HS
                qdma = dma if hn == 0 else nc.gpsimd.dma_start
                if part in ("both", "kt"):
                    for h in range(HPG):
                        d0 = h * N + hs0
                        r = slice(24 * h, 24 * h + 24)
                        eng = nc.gpsimd.dma_start if (hn == 1 and h % 2) else dma
                        eng(kt16[0:24, d0:d0 + HS], erk[r, :])
                        eng(kt16[24:48, d0:d0 + HS], ork[r, :])
                        eng(kt16[48:72, d0:d0 + HS], prk[r, :])
                if part in ("both", "qt"):
                    for h in range(HPG):
                        d0 = h * N + hs0
                        r = slice(24 * h, 24 * h + 24)
                        qdma(qt16[0:24, d0:d0 + HS], erq[r, :])
                        qdma(qt16[24:48, d0:d0 + HS], orq[r, :])
                        qdma(qt16[48:72, d0:d0 + HS], prq[r, :])

            def emit_v_tt(qn, tt, ps_pool, ks=range(CK), box=None):
                """V for all 4 heads, one 128-token tile, x-stationary.
                ks selects the contraction slice so fillers can split the
                accumulation into small units (box carries the psum tile)."""
                ts0 = qn * QS
                if box is None:
                    box = {}
                if "vp" not in box:
                    box["vp"] = ps_pool.tile([128, QS], F32, tag="op", bufs=2,
                                             name=f"vps{qn}_{tt}")
                vp = box["vp"]
                for k in ks:
                    nc.tensor.matmul(
                        vp[:, 0:HPG * HD],
                        x16[k][:, ts0 + tt * 128:ts0 + (tt + 1) * 128],
                        wqk_t[k][:, 576:576 + HPG * HD],
                        start=(k == 0), stop=(k == CK - 1),
                    )
                if ks[-1] == CK - 1:
                    cp = nc.scalar.copy if qn < 2 else nc.vector.tensor_copy
                    cp(
                        v16[qn * 4 + tt][:, :, 0:HD],
                        vp[:, 0:HPG * HD].rearrange("p (h d) -> p h d", h=HPG),
                    )

            def emit_proj(ct, jq, ps_pool, sb_pool):
                op = ps_pool.tile([128, QS], F32, tag="op", bufs=2,
                                  name=f"op{ct}_{jq}")
                for i in range(HPG):
                    hd2 = HD + 1 if i == 3 else HD
                    nc.tensor.matmul(
                        op[:], wp4[0:hd2, i, ct * 128:(ct + 1) * 128],
                        o16[i][:, jq * QS:(jq + 1) * QS],
                        start=(i == 0), stop=(i == HPG - 1),
                    )
                osb = sb_pool.tile([128, QS], F32, tag="osb", bufs=3,
                                   name=f"osb{ct}_{jq}")
                # copies alternate DVE/Pool; out-DMA issues from SP (the ACT
                # sequencer is saturated with exps in phase 2)
                if ct % 2 == 0:
                    nc.gpsimd.tensor_copy(osb[:], op[:])
                else:
                    nc.vector.tensor_copy(osb[:], op[:])
                nc.sync.dma_start(
                    outT[ct * 128:(ct + 1) * 128, jq * QS:(jq + 1) * QS], osb[:]
                )

            # ================= phase 1: QKV + RoPE + repack =================
            _s2cm = tc.tile_pool(name="s2", bufs=1)
            s2 = _s2cm.__enter__()
            early_es = {}

            def emit_partA_kp(h, kp):
                """Scores+exp for one kp of chunk (h, jq=0), emitted inside
                phase 1 once the half-0 repack is in flight. Uses two [128,QS]
                PSUM tiles from the shared 'op' tag and f512 exps so no extra
                banks are needed."""
                hb = h * N
                if True:
                    sts = []
                    for i in range(2):
                        kt = 2 * kp + i
                        stx = ps1.tile([128, QS], F32, tag="op", bufs=2,
                                       name=f"stE{h}_{kp}_{i}")
                        nc.tensor.matmul(
                            stx[:],
                            kt16[:, hb + kt * 128:hb + (kt + 1) * 128],
                            qt16[:, hb:hb + QS],
                            start=True, stop=True,
                        )
                        sts.append(stx)
                    e = s2.tile([128, 2 * QS], F16, tag="e", bufs=23,
                                name=f"eE{h}_{kp}")
                    for i in range(2):
                        nc.scalar.activation(e[:, i * QS:(i + 1) * QS],
                                             sts[i][:], AF.Exp, scale=SCALE)
                    early_es.setdefault(h, []).append(e)

            def emit_partA(h, ps_pool, between=None):
                for kp in range(4):
                    if between is not None:
                        between()
                    emit_partA_kp(h, kp)

            with (
                tc.tile_pool(name="s1", bufs=1) as s1,
                tc.tile_pool(name="ps1", bufs=1, space="PSUM") as ps1,
            ):
                halves = [
                    tuple(
                        s1.tile([96, HS], F16, tag=f"{nm}", bufs=1,
                                name=f"{nm}_{hn}")
                        for nm in ("erq", "orq", "prq", "erk", "ork", "prk")
                    )
                    for hn in range(2)
                ]
                kp_ctr = [0]

                def hook():
                    if kp_ctr[0] < 4:
                        emit_partA_kp(0, kp_ctr[0])
                        kp_ctr[0] += 1

                for qn in range(4):
                    emit_qkrope(ps1, s1, qn, halves,
                                hook=hook if qn == 3 else None)
                    if qn == 1:
                        for tt in range(4):
                            emit_v_tt(0, tt, ps1)
                        emit_repack(0, halves)
                    if qn == 2:
                        for tt in range(4):
                            emit_v_tt(1, tt, ps1)
                    if qn == 3:
                        emit_repack(1, halves, part="kt")
                        emit_partA(1, ps1)
                        emit_repack(1, halves, part="qt")

            # ================= phase 2: attention + projection ===============
            with tc.tile_pool(name="ps2", bufs=1, space="PSUM") as ps2:
                fillers = deque()
                for qn in (2, 3):
                    for tt in range(4):
                        vbox = {}
                        for ks in (range(0, 3), range(3, 6), range(6, CK)):
                            fillers.append((288 * len(ks),
                                            lambda qn=qn, tt=tt, ks=ks, vbox=vbox:
                                            emit_v_tt(qn, tt, ps2, ks, vbox)))

                def mk_proj_filler(ct, jq):
                    # two units: heads 0-1, then heads 2-3 + copy + store
                    pbox = {}

                    def a():
                        pbox["op"] = ps2.tile([128, QS], F32, tag="op", bufs=2,
                                              name=f"op{ct}_{jq}")
                        for i in (0, 1):
                            nc.tensor.matmul(
                                pbox["op"][:],
                                wp4[0:HD, i, ct * 128:(ct + 1) * 128],
                                o16[i][:, jq * QS:(jq + 1) * QS],
                                start=(i == 0), stop=False,
                            )
                        return 1024

                    def b():
                        op = pbox["op"]
                        for i in (2, 3):
                            hd2 = HD + 1 if i == 3 else HD
                            nc.tensor.matmul(
                                op[:], wp4[0:hd2, i, ct * 128:(ct + 1) * 128],
                                o16[i][:, jq * QS:(jq + 1) * QS],
                                start=False, stop=(i == 3),
                            )
                        osb = s2.tile([128, QS], F32, tag="osb", bufs=3,
                                      name=f"osb{ct}_{jq}")
                        nc.vector.tensor_copy(osb[:], op[:])
                        nc.sync.dma_start(
                            outT[ct * 128:(ct + 1) * 128,
                                 jq * QS:(jq + 1) * QS], osb[:]
                        )
                        return 1664
                    return [(1024, a), (1664, b)]

                def mk_proj_ab(ct, jq):
                    pbox = {}

                    def a():
                        op = ps2.tile([128, QS], F32, tag="op", bufs=2,
                                      name=f"opA{ct}_{jq}")
                        for i in (0, 1):
                            nc.tensor.matmul(
                                op[:], wp4[0:HD, i, ct * 128:(ct + 1) * 128],
                                o16[i][:, jq * QS:(jq + 1) * QS],
                                start=(i == 0), stop=(i == 1),
                            )
                        park = s2.tile([128, QS], F16, tag="park", bufs=9,
                                       name=f"park{ct}")
                        nc.vector.tensor_copy(park[:], op[:])
                        pbox["park"] = park
                        return 1024

                    def b():
                        op = ps2.tile([128, QS], F32, tag="op", bufs=2,
                                      name=f"opB{ct}_{jq}")
                        for i in (2, 3):
                            hd2 = HD + 1 if i == 3 else HD
                            nc.tensor.matmul(
                                op[:], wp4[0:hd2, i, ct * 128:(ct + 1) * 128],
                                o16[i][:, jq * QS:(jq + 1) * QS],
                                start=(i == 2), stop=(i == 3),
                            )
                        osb = s2.tile([128, QS], F32, tag="osb", bufs=3,
                                      name=f"osb{ct}_{jq}")
                        nc.vector.tensor_tensor(osb[:], op[:],
                                                pbox["park"][:], ALU.add)
                        nc.sync.dma_start(
                            outT[ct * 128:(ct + 1) * 128,
                                 jq * QS:(jq + 1) * QS], osb[:]
                        )
                        return 1664
                    return (1024, a), (1664, b)

                def drain(budget):
                    while fillers and budget > 0:
                        cost, fn = fillers.popleft()
                        fn()
                        budget -= cost

                def emit_scores(h, jq, first=False):
                    hb = h * N
                    es = []
                    kps = range(8)
                    if jq == 0 and h in early_es:
                        es = list(early_es[h])
                        kps = range(4, 8)
                    for kp in kps:
                        st = ps2.tile([128, 2 * QS], F32, tag="st", bufs=2,
                                      name=f"st{h}_{jq}_{kp}")
                        for i in range(2):
                            kt = 2 * kp + i
                            nc.tensor.matmul(
                                st[:, i * QS:(i + 1) * QS],
                                kt16[:, hb + kt * 128:hb + (kt + 1) * 128],
                                qt16[:, hb + jq * QS:hb + (jq + 1) * QS],
                                start=True, stop=True,
                            )
                        e = s2.tile([128, 2 * QS], F16, tag="e", bufs=23,
                                    name=f"e{h}_{jq}_{kp}")
                        nc.scalar.activation(e[:], st[:], AF.Exp, scale=SCALE)
                        es.append(e)
                        drain(700 if kp == kps[0] else 1300)
                    return es

                av_ready = deque()

                def tail_units(h, jq, es):
                    """AV + normalize + transpose for a finished chunk, as filler units."""
                    units = []
                    box = {}

                    def mk_av(kp):
                        def f():
                            if kp == 0:
                                if av_ready:
                                    box["av"] = av_ready.popleft()
                                else:
                                    box["av"] = ps2.tile(
                                        [128, NQ, HD + 1], F32, tag="av",
                                        bufs=1, name=f"av{h}_{jq}")
                                    # 4 accumulation regions share one PSUM
                                    # bank; a start=True zeroes the whole
                                    # bank, so zero once, accumulate with
                                    # start=False
                                    nc.vector.memset(box["av"][:], 0.0)
                            av = box["av"]
                            for i in range(2):
                                kt = 2 * kp + i
                                for qi in range(NQ):
                                    nc.tensor.matmul(
                                        av[:, qi, :],
                                        es[kp][:, i * QS + qi * 128:
                                               i * QS + (qi + 1) * 128],
                                        v16[kt][:, h, :],
                                        start=False, stop=(kt == KTILES - 1),
                                        skip_group_check=True,
                                    )
                            return 584
                        return f

                    for kp in range(8):
                        units.append((584, mk_av(kp)))

                    def f_fin():
                        av = box["av"]
                        rec = s2.tile([128, NQ], F32, tag="rec", bufs=2,
                                      name=f"rec{h}_{jq}")
                        nc.vector.reciprocal(rec[:], av[:, :, HD])
                        o_n = s2.tile([128, NQ, HD], F16, tag="on", bufs=2,
                                      name=f"on{h}_{jq}")
                        for qi in range(NQ):
                            nc.vector.tensor_scalar_mul(
                                o_n[:, qi, :], av[:, qi, 0:HD], rec[:, qi:qi + 1]
                            )
                        nxt = ps2.tile([128, NQ, HD + 1], F32, tag="av",
                                       bufs=1, name=f"avn{h}_{jq}")
                        nc.vector.memset(nxt[:], 0.0)
                        av_ready.append(nxt)
                        pt = ps2.tile([HD, NQ, 128], F16, tag="pt", bufs=1,
                                      name=f"pt{h}_{jq}")
                        for qi in range(NQ):
                            nc.tensor.transpose(pt[:, qi, :], o_n[:, qi, :],
                                                ident[:])
                        nc.vector.tensor_copy(
                            o16[h][0:HD, jq * QS:(jq + 1) * QS], pt[:]
                        )
                        return 512
                    units.append((512, f_fin))
                    return units

                ab_units = [mk_proj_ab(ct, NQ - 1) for ct in range(CK)]
                prev = None
                for jq in range(NQ):
                    for h in range(HPG):
                        if prev is not None:
                            fillers.extend(tail_units(*prev))
                            if prev[0] == 3 and prev[1] < NQ - 1:
                                for ct in range(CK):
                                    fillers.extend(mk_proj_filler(ct, prev[1]))
                            if prev[0] == 1 and prev[1] == NQ - 1:
                                fillers.extend(u[0] for u in ab_units)
                        es = emit_scores(h, jq, first=(jq == 0 and h == 0))
                        prev = (h, jq, es)
                fillers.extend(tail_units(*prev))
                fillers.extend(u[1] for u in ab_units)
                drain(10 ** 9)
            _s2cm.__exit__(None, None, None)

    bass_rust.generate_event_semaphores(nc)
    return nc


_NC = None


def _get_nc():
    global _NC
    if _NC is None:
        _NC = build_nc()
    return _NC


def kernel(x, Wqkv, Wproj, bproj, T, H, W):
    x = np.asarray(x, dtype=np.float32)
    Wqkv = np.asarray(Wqkv, dtype=np.float32)
    Wproj = np.asarray(Wproj, dtype=np.float32)
    bproj = np.asarray(bproj, dtype=np.float32)
    assert x.shape == (B, N, C) and Wqkv.shape == (C, 3 * C)
    assert (int(T), int(H), int(W)) == (GT, GH, GW)

    cos96, sin96 = _cos_sin_96()
    nc = _get_nc()

    in_maps = []
    for core in range(NCORES):
        b, g = divmod(core, HPG)
        heads = [HPG * g + i for i in range(HPG)]
        qe = [h * HD + 2 * j for h in heads for j in range(24)]
        qo = [h * HD + 2 * j + 1 for h in heads for j in range(24)]
        qp = [h * HD + 48 + j for h in heads for j in range(24)]
        ke = [C + i for i in qe]
        ko = [C + i for i in qo]
        kp = [C + i for i in qp]
        vcols = [2 * C + h * HD + d for h in heads for d in range(HD)]
        cols = (qe + qp[0:32] + qo + qp[32:64] + ke + qp[64:96]
                + ko + kp[0:32] + kp[32:96] + vcols)
        wqk_c = Wqkv[:, cols].astype(np.float16)
        bias_row = bproj if g == 0 else np.zeros_like(bproj)
        # wp packed [73, HPG*C]: head h columns [h*C:(h+1)*C], row 72 = bias
        # (only meaningful for h==3, whose o16 carries the ones row)
        wp_c = np.zeros((HD + 1, HPG * C), dtype=np.float32)
        for i, h in enumerate(heads):
            wp_c[0:HD, i * C:(i + 1) * C] = Wproj[h * HD:(h + 1) * HD, :]
        wp_c[HD, 3 * C:4 * C] = bias_row
        in_maps.append({
            "xT": np.ascontiguousarray(x[b].T).astype(np.float16),
            "wqk": np.ascontiguousarray(wqk_c),
            "wp": wp_c.astype(np.float16),
            "cosd": cos96.astype(np.float16),
            "sind": sin96.astype(np.float16),
        })

    res = run_bass_kernel_spmd(nc, in_maps, core_ids=list(range(NCORES)))
    out = np.zeros((B, N, C), dtype=np.float32)
    for core in range(NCORES):
        b = core // HPG
        out[b] += res.results[core]["outT"].T
    return out


# revision 43
# speedup vs baseline: 1.0110x; 1.0110x over previous
"""Trainium2 Bass kernel for nn_Attention_79224966742132.

Dense transformer attention block: QKV projection + axial RoPE + SDPA +
output projection, for x (2, 2048, 1152), 16 heads of dim 72.

Sharding (8 cores): data-parallel over batch (2) x tensor-parallel over
head groups (4 heads/core). Each core computes QKV for its 4 heads from
the full x[b], applies RoPE, runs attention, and produces a partial
output projection (row-parallel Wproj); the host sums the 4 partials per
batch element. The projection bias rides as an extra contraction row on
the g==0 core of each batch.

v3 design notes (against the TimelineSim cost model):
- All phase-1 matmuls in fp16 (1 cycle/row at any moving size); x, Wqkv,
  Wv are quantized to fp16 on the host (~1e-3 rel err, gate is 2e-2).
- QK projection packed into 5 stationary blocks of <=128 columns
  (4x128 + 64) instead of 6x96: pass-dims fill the block remainders.
- Attention-value matmul restructured: exp-weights tile [128kt, 128qt]
  is the STATIONARY operand, v [128, 73] fp16 the moving one -> 73
  cycles per k-tile instead of 512 (output lands as [qtok, hd]; a cheap
  PE transpose brings it back to [hd, qtok] for the projection).
- Softmax denominator = ones column appended to v; reciprocal + scale on
  DVE in the [qtok, hd] layout (per-partition scalar, no broadcasts).
- Projection bias folded into the h3 projection matmul as a 73rd
  contraction row against a ones row in o16[3].
- The exp on ACT (133us) is the phase-2 near-critical path; V for token
  halves 2-3, all projection matmuls, and each chunk's AV/normalize/
  transpose tail are deferred into a filler queue drained between score
  matmuls so PE never idles while ACT catches up.
- Half-0 scores+exps for chunks (h0,jq0)/(h1,jq0) are emitted inside
  phase 1 (hooked between qn3's blocks) so ACT starts ~20us early.
- HWDGE descriptor-gen is a single serialized device (~625ns/DMA) and
  DMA transfers serialize on one modeled DMA_ENGINES device: DMA count
  is minimized (wv merged into the wqk tensor, wp packed into one
  tile); latency-critical repacks go HWDGE, the rest SWDGE (Pool).
- PSUM accumulation groups are zeroed per BANK by matmul start=True, so
  the four AV sub-tile groups sharing one bank are memset once and
  accumulated with start=False.
- GPSIMD cannot touch PSUM and engine APs need 32-aligned partition
  offsets (hardware rules the cost model does not check).
"""
import math
import os
import sys
from collections import deque

# The device path needs the axon/neuron jax platform; if a harness pinned
# JAX_PLATFORMS=cpu (common for running jax references) and jax is not yet
# imported, restore platform auto-detection.
if "jax" not in sys.modules:
    _jp = os.environ.get("JAX_PLATFORMS")
    if _jp and "axon" not in _jp and "neuron" not in _jp:
        del os.environ["JAX_PLATFORMS"]

import numpy as np

import bass_rust
import concourse.bass as bass
import concourse.mybir as mybir
import concourse.tile as tile
from concourse.bass_utils import run_bass_kernel_spmd
from concourse.masks import make_identity

F32 = mybir.dt.float32
F16 = mybir.dt.float16
AF = mybir.ActivationFunctionType
ALU = mybir.AluOpType

B = 2
N = 2048          # tokens = T*H*W = 8*16*16
C = 1152
NH = 16
HD = 72
HPG = 4           # heads per core
NCORES = 8
GT, GH, GW = 8, 16, 16
SCALE = 1.0 / math.sqrt(HD)

NQ = 4            # q-chunks (512 tokens each) and qt-subtiles per chunk
QS = N // NQ      # 512
KTILES = N // 128  # 16
CK = C // 128      # 9 contraction chunks
HS = N // 2        # RoPE-output/repack half granularity


def _axis_freqs(n: int) -> np.ndarray:
    base = np.linspace(1.0, 128.0, 8, dtype=np.float64) * np.pi   # MAX_FREQ/2
    pos = np.linspace(-1.0, 1.0, n, dtype=np.float64)
    return pos[:, None] * base[None, :]                            # (n, 8)


def _cos_sin_96():
    """cos/sin of the 24 pair frequencies per token, tiled x4 -> (96, N)."""
    f = np.zeros((GT, GH, GW, 24), dtype=np.float64)
    f[..., 0:8] = _axis_freqs(GT)[:, None, None, :]
    f[..., 8:16] = _axis_freqs(GH)[None, :, None, :]
    f[..., 16:24] = _axis_freqs(GW)[None, None, :, :]
    f = f.reshape(N, 24)
    cos24 = np.ascontiguousarray(np.cos(f).astype(np.float32).T)   # (24, N)
    sin24 = np.ascontiguousarray(np.sin(f).astype(np.float32).T)
    return np.tile(cos24, (4, 1)), np.tile(sin24, (4, 1))          # (96, N)


def build_nc() -> bass.Bass:
    nc = bass.Bass()
    xT = nc.dram_tensor("xT", [C, N], F16, kind="ExternalInput")
    wqk = nc.dram_tensor("wqk", [C, 576 + HPG * HD], F16, kind="ExternalInput")
    wp = nc.dram_tensor("wp", [HD + 1, HPG * C], F16, kind="ExternalInput")
    cosd = nc.dram_tensor("cosd", [96, N], F16, kind="ExternalInput")
    sind = nc.dram_tensor("sind", [96, N], F16, kind="ExternalInput")
    outT = nc.dram_tensor("outT", [C, N], F32, kind="ExternalOutput")

    with tile.TileContext(nc) as tc:
        with tc.tile_pool(name="persist", bufs=1) as pp:
            x16 = [pp.tile([128, N], F16, name=f"x16_{k}") for k in range(CK)]
            qt16 = pp.tile([HD, HPG * N], F16, name="qt16")
            kt16 = pp.tile([HD, HPG * N], F16, name="kt16")
            v16 = [pp.tile([128, HPG, HD + 1], F16, name=f"v16_{i}")
                   for i in range(KTILES)]
            o16 = [pp.tile([HD + (1 if h == 3 else 0), N], F16, name=f"o16_{h}")
                   for h in range(HPG)]
            wqk_t = [pp.tile([128, 576 + HPG * HD], F16, name=f"wqk{k}")
                     for k in range(CK)]
            wp4 = pp.tile([HD + 1, HPG, C], F16, name="wp4")
            cos_t = pp.tile([96, N], F16, name="cos_t")
            sin_t = pp.tile([96, N], F16, name="sin_t")
            ident = pp.tile([128, 128], F16, name="ident")

            ones_row = pp.tile([1, N], F16, name="ones_row")
            make_identity(nc, ident[:])
            for i in range(KTILES):
                nc.vector.memset(v16[i][:, :, HD], 1.0)
            nc.vector.memset(ones_row[:], 1.0)
            # engine writes need 32-aligned partition offsets; DMA does not
            nc.sync.dma_start(o16[3][HD:HD + 1, :], ones_row[:])

            # bulk loads alternate between the two HWDGE issuers (SP + ACT,
            # 16 queues each) so transfers run in parallel and neither
            # sequencer serializes the load phase
            _eng = [nc.sync, nc.scalar]
            _ei = [0]

            def dma(out, in_):
                _eng[_ei[0] & 1].dma_start(out, in_)
                _ei[0] += 1

            for k in range(CK):
                dma(wqk_t[k][:], wqk[k * 128:(k + 1) * 128, :])
                dma(x16[k][:, 0:HS], xT[k * 128:(k + 1) * 128, 0:HS])
            dma(cos_t[:], cosd[:, :])
            dma(sin_t[:], sind[:, :])
            for k in range(CK):
                dma(x16[k][:, HS:N], xT[k * 128:(k + 1) * 128, HS:N])
            dma(wp4[:], wp[:].rearrange("p (h c) -> p h c", h=HPG))

            # ---------------- emit helpers ----------------

            def emit_qkrope(ps_pool, sb_pool, qn, halves, hook=None):
                """5-block QK matmuls + RoPE for one token quarter, 4 heads.

                Column blocks (stationary, host-packed):
                  B0 = Qe(96) + Qp[0:32]     B1 = Qo(96) + Qp[32:64]
                  B2 = Ke(96) + Qp[64:96]    B3 = Ko(96) + Kp[0:32]
                  B4 = Kp[32:96]
                where e/o/p = rotary-even/odd/pass dims, head-major.
                RoPE for Q is emitted right after B1 (and K after B3) so the
                DVE chain starts early and single-buffered PSUM blocks never
                stall the next quarter.
                """
                ts0 = qn * QS
                hn, sub = divmod(qn, 2)
                sl = slice(sub * QS, (sub + 1) * QS)
                erq, orq, prq, erk, ork, prk = halves[hn]
                cosq = cos_t[:, ts0:ts0 + QS]
                sinq = sin_t[:, ts0:ts0 + QS]

                def mm_block(m):
                    w = 64 if m == 4 else 128
                    blk = ps_pool.tile([w, QS], F32, tag=f"qk{m}", bufs=1,
                                       name=f"qk{qn}_{m}")
                    for k in range(CK):
                        nc.tensor.matmul(
                            blk[:],
                            wqk_t[k][:, 128 * m:128 * m + w],
                            x16[k][:, ts0:ts0 + QS],
                            start=(k == 0), stop=(k == CK - 1),
                        )
                    return blk

                def rope(e_blk, o_blk, er, orr):
                    t1 = sb_pool.tile([96, QS], F16, tag="rtA", bufs=1,
                                      name=f"t1_{qn}")
                    t2 = sb_pool.tile([96, QS], F16, tag="rtB", bufs=1,
                                      name=f"t2_{qn}")
                    nc.vector.tensor_tensor(t1[:], e_blk[0:96, :], cosq, ALU.mult)
                    nc.vector.tensor_tensor(t2[:], o_blk[0:96, :], sinq, ALU.mult)
                    nc.vector.tensor_tensor(er[:, sl], t1[:], t2[:], ALU.subtract)
                    t3 = sb_pool.tile([96, QS], F16, tag="rtA", bufs=1,
                                      name=f"t3_{qn}")
                    t4 = sb_pool.tile([96, QS], F16, tag="rtB", bufs=1,
                                      name=f"t4_{qn}")
                    nc.vector.tensor_tensor(t3[:], o_blk[0:96, :], cosq, ALU.mult)
                    nc.vector.tensor_tensor(t4[:], e_blk[0:96, :], sinq, ALU.mult)
                    nc.vector.tensor_tensor(orr[:, sl], t3[:], t4[:], ALU.add)

                if qn == 3:
                    # K first: the half-1 kt repack gates phase 2
                    B2 = mm_block(2)
                    if hook: hook()
                    B3 = mm_block(3)
                    rope(B2, B3, erk, ork)
                    if hook: hook()
                    B0 = mm_block(0)
                    if hook: hook()
                    B1 = mm_block(1)
                    rope(B0, B1, erq, orq)
                    if hook: hook()
                else:
                    B0 = mm_block(0)
                    B1 = mm_block(1)
                    rope(B0, B1, erq, orq)
                    B2 = mm_block(2)
                    B3 = mm_block(3)
                    rope(B2, B3, erk, ork)
                B4 = mm_block(4)
                # pass dims: Qp spread over B0/B1/B2 remainders, Kp over B3/B4.
                # The last quarter's copies go to the (idle) ACT engine so the
                # PSUM banks free up fast for phase 2.
                nc.scalar.copy(prq[0:32, sl], B0[96:128, :])
                nc.scalar.copy(prq[32:64, sl], B1[96:128, :])
                nc.scalar.copy(prq[64:96, sl], B2[96:128, :])
                nc.scalar.copy(prk[0:32, sl], B3[96:128, :])
                nc.scalar.copy(prk[32:64, sl], B4[0:32, :])
                nc.scalar.copy(prk[64:96, sl], B4[32:64, :])

            def emit_repack(hn, halves, part="both", quarter=None):
                """DMA the rotated halves into per-head [72, N] q/k tiles.

                Per-head dim order: [0:24] even-rotated, [24:48] odd-rotated,
                [48:72] pass -- same permutation for q and k, so scores are
                unchanged. Issues alternate between the two HWDGE engines.
                """
                erq, orq, prq, erk, ork, prk = halves[hn]
                if quarter is None:
                    cs, sz = slice(0, HS), HS
                else:
                    cs, sz = slice(quarter * QS, (quarter + 1) * QS), QS
                hs0 = hn * HS + (0 if quarter is None else quarter * QS)
                qdma = dma if hn == 0 else nc.gpsimd.dma_start
                if part in ("both", "kt"):
                    for h in range(HPG):
                        d0 = h * N + hs0
                        r = slice(24 * h, 24 * h + 24)
                        eng = nc.gpsimd.dma_start if (hn == 1 and h % 2) else dma
                        eng(kt16[0:24, d0:d0 + sz], erk[r, cs])
                        eng(kt16[24:48, d0:d0 + sz], ork[r, cs])
                        eng(kt16[48:72, d0:d0 + sz], prk[r, cs])
                if part in ("both", "qt"):
                    for h in range(HPG):
                        d0 = h * N + hs0
                        r = slice(24 * h, 24 * h + 24)
                        qdma(qt16[0:24, d0:d0 + sz], erq[r, cs])
                        qdma(qt16[24:48, d0:d0 + sz], orq[r, cs])
                        qdma(qt16[48:72, d0:d0 + sz], prq[r, cs])

            def emit_v_tt(qn, tt, ps_pool, ks=range(CK), box=None):
                """V for all 4 heads, one 128-token tile, x-stationary.
                ks selects the contraction slice so fillers can split the
                accumulation into small units (box carries the psum tile)."""
                ts0 = qn * QS
                if box is None:
                    box = {}
                if "vp" not in box:
                    box["vp"] = ps_pool.tile([128, QS], F32, tag="op", bufs=2,
                                             name=f"vps{qn}_{tt}")
                vp = box["vp"]
                for k in ks:
                    nc.tensor.matmul(
                        vp[:, 0:HPG * HD],
                        x16[k][:, ts0 + tt * 128:ts0 + (tt + 1) * 128],
                        wqk_t[k][:, 576:576 + HPG * HD],
                        start=(k == 0), stop=(k == CK - 1),
                    )
                if ks[-1] == CK - 1:
                    cp = nc.scalar.copy if qn < 2 else nc.vector.tensor_copy
                    cp(
                        v16[qn * 4 + tt][:, :, 0:HD],
                        vp[:, 0:HPG * HD].rearrange("p (h d) -> p h d", h=HPG),
                    )

            def emit_proj(ct, jq, ps_pool, sb_pool):
                op = ps_pool.tile([128, QS], F32, tag="op", bufs=2,
                                  name=f"op{ct}_{jq}")
                for i in range(HPG):
                    hd2 = HD + 1 if i == 3 else HD
                    nc.tensor.matmul(
                        op[:], wp4[0:hd2, i, ct * 128:(ct + 1) * 128],
                        o16[i][:, jq * QS:(jq + 1) * QS],
                        start=(i == 0), stop=(i == HPG - 1),
                    )
                osb = sb_pool.tile([128, QS], F32, tag="osb", bufs=3,
                                   name=f"osb{ct}_{jq}")
                # copies alternate DVE/Pool; out-DMA issues from SP (the ACT
                # sequencer is saturated with exps in phase 2)
                if ct % 2 == 0:
                    nc.gpsimd.tensor_copy(osb[:], op[:])
                else:
                    nc.vector.tensor_copy(osb[:], op[:])
                nc.sync.dma_start(
                    outT[ct * 128:(ct + 1) * 128, jq * QS:(jq + 1) * QS], osb[:]
                )

            # ================= phase 1: QKV + RoPE + repack =================
            _s2cm = tc.tile_pool(name="s2", bufs=1)
            s2 = _s2cm.__enter__()
            early_es = {}

            def emit_partA_kp(h, kp):
                """Scores+exp for one kp of chunk (h, jq=0), emitted inside
                phase 1 once the half-0 repack is in flight. Uses two [128,QS]
                PSUM tiles from the shared 'op' tag and f512 exps so no extra
                banks are needed."""
                hb = h * N
                if True:
                    sts = []
                    for i in range(2):
                        kt = 2 * kp + i
                        stx = ps1.tile([128, QS], F32, tag="op", bufs=2,
                                       name=f"stE{h}_{kp}_{i}")
                        nc.tensor.matmul(
                            stx[:],
                            kt16[:, hb + kt * 128:hb + (kt + 1) * 128],
                            qt16[:, hb:hb + QS],
                            start=True, stop=True,
                        )
                        sts.append(stx)
                    e = s2.tile([128, 2 * QS], F16, tag="e", bufs=23,
                                name=f"eE{h}_{kp}")
                    for i in range(2):
                        nc.scalar.activation(e[:, i * QS:(i + 1) * QS],
                                             sts[i][:], AF.Exp, scale=SCALE)
                    early_es.setdefault(h, []).append(e)

            def emit_partA(h, ps_pool, between=None):
                for kp in range(4):
                    if between is not None:
                        between()
                    emit_partA_kp(h, kp)

            with (
                tc.tile_pool(name="s1", bufs=1) as s1,
                tc.tile_pool(name="ps1", bufs=1, space="PSUM") as ps1,
            ):
                halves = [
                    tuple(
                        s1.tile([96, HS], F16, tag=f"{nm}", bufs=1,
                                name=f"{nm}_{hn}")
                        for nm in ("erq", "orq", "prq", "erk", "ork", "prk")
                    )
                    for hn in range(2)
                ]
                kp_ctr = [0]

                def hook():
                    if kp_ctr[0] < 4:
                        emit_partA_kp(0, kp_ctr[0])
                        kp_ctr[0] += 1

                for qn in range(4):
                    emit_qkrope(ps1, s1, qn, halves,
                                hook=hook if qn == 3 else None)
                    if qn == 1:
                        for tt in range(4):
                            emit_v_tt(0, tt, ps1)
                        emit_repack(0, halves)
                    if qn == 2:
                        for tt in range(4):
                            emit_v_tt(1, tt, ps1)
                        emit_repack(1, halves, part="kt", quarter=0)
                    if qn == 3:
                        emit_repack(1, halves, part="kt", quarter=1)
                        emit_partA(1, ps1)
                        emit_repack(1, halves, part="qt")

            # ================= phase 2: attention + projection ===============
            with tc.tile_pool(name="ps2", bufs=1, space="PSUM") as ps2:
                fillers = deque()
                for qn in (2, 3):
                    for tt in range(4):
                        vbox = {}
                        for ks in (range(0, 3), range(3, 6), range(6, CK)):
                            fillers.append((288 * len(ks),
                                            lambda qn=qn, tt=tt, ks=ks, vbox=vbox:
                                            emit_v_tt(qn, tt, ps2, ks, vbox)))

                def mk_proj_filler(ct, jq):
                    # two units: heads 0-1, then heads 2-3 + copy + store
                    pbox = {}

                    def a():
                        pbox["op"] = ps2.tile([128, QS], F32, tag="op", bufs=2,
                                              name=f"op{ct}_{jq}")
                        for i in (0, 1):
                            nc.tensor.matmul(
                                pbox["op"][:],
                                wp4[0:HD, i, ct * 128:(ct + 1) * 128],
                                o16[i][:, jq * QS:(jq + 1) * QS],
                                start=(i == 0), stop=False,
                            )
                        return 1024

                    def b():
                        op = pbox["op"]
                        for i in (2, 3):
                            hd2 = HD + 1 if i == 3 else HD
                            nc.tensor.matmul(
                                op[:], wp4[0:hd2, i, ct * 128:(ct + 1) * 128],
                                o16[i][:, jq * QS:(jq + 1) * QS],
                                start=False, stop=(i == 3),
                            )
                        osb = s2.tile([128, QS], F32, tag="osb", bufs=3,
                                      name=f"osb{ct}_{jq}")
                        nc.vector.tensor_copy(osb[:], op[:])
                        nc.sync.dma_start(
                            outT[ct * 128:(ct + 1) * 128,
                                 jq * QS:(jq + 1) * QS], osb[:]
                        )
                        return 1664
                    return [(1024, a), (1664, b)]

                def mk_proj_ab(ct, jq):
                    pbox = {}

                    def a():
                        op = ps2.tile([128, QS], F32, tag="op", bufs=2,
                                      name=f"opA{ct}_{jq}")
                        for i in (0, 1):
                            nc.tensor.matmul(
                                op[:], wp4[0:HD, i, ct * 128:(ct + 1) * 128],
                                o16[i][:, jq * QS:(jq + 1) * QS],
                                start=(i == 0), stop=(i == 1),
                            )
                        park = s2.tile([128, QS], F16, tag="park", bufs=9,
                                       name=f"park{ct}")
                        nc.vector.tensor_copy(park[:], op[:])
                        pbox["park"] = park
                        return 1024

                    def b():
                        op = ps2.tile([128, QS], F32, tag="op", bufs=2,
                                      name=f"opB{ct}_{jq}")
                        for i in (2, 3):
                            hd2 = HD + 1 if i == 3 else HD
                            nc.tensor.matmul(
                                op[:], wp4[0:hd2, i, ct * 128:(ct + 1) * 128],
                                o16[i][:, jq * QS:(jq + 1) * QS],
                                start=(i == 2), stop=(i == 3),
                            )
                        osb = s2.tile([128, QS], F32, tag="osb", bufs=3,
                                      name=f"osb{ct}_{jq}")
                        nc.vector.tensor_tensor(osb[:], op[:],
                                                pbox["park"][:], ALU.add)
                        nc.sync.dma_start(
                            outT[ct * 128:(ct + 1) * 128,
                                 jq * QS:(jq + 1) * QS], osb[:]
                        )
                        return 1664
                    return (1024, a), (1664, b)

                def drain(budget):
                    while fillers and budget > 0:
                        cost, fn = fillers.popleft()
                        fn()
                        budget -= cost

                def emit_scores(h, jq, first=False):
                    hb = h * N
                    es = []
                    kps = range(8)
                    if jq == 0 and h in early_es:
                        es = list(early_es[h])
                        kps = range(4, 8)
                    for kp in kps:
                        st = ps2.tile([128, 2 * QS], F32, tag="st", bufs=2,
                                      name=f"st{h}_{jq}_{kp}")
                        for i in range(2):
                            kt = 2 * kp + i
                            nc.tensor.matmul(
                                st[:, i * QS:(i + 1) * QS],
                                kt16[:, hb + kt * 128:hb + (kt + 1) * 128],
                                qt16[:, hb + jq * QS:hb + (jq + 1) * QS],
                                start=True, stop=True,
                            )
                        e = s2.tile([128, 2 * QS], F16, tag="e", bufs=23,
                                    name=f"e{h}_{jq}_{kp}")
                        nc.scalar.activation(e[:], st[:], AF.Exp, scale=SCALE)
                        es.append(e)
                        drain(0 if kp < kps[0] + 2 else 2100)
                    return es

                av_ready = deque()

                def tail_units(h, jq, es):
                    """AV + normalize + transpose for a finished chunk, as filler units."""
                    units = []
                    box = {}

                    def mk_av(kp):
                        def f():
                            if kp == 0:
                                if av_ready:
                                    box["av"] = av_ready.popleft()
                                else:
                                    box["av"] = ps2.tile(
                                        [128, NQ, HD + 1], F32, tag="av",
                                        bufs=1, name=f"av{h}_{jq}")
                                    # 4 accumulation regions share one PSUM
                                    # bank; a start=True zeroes the whole
                                    # bank, so zero once, accumulate with
                                    # start=False
                                    nc.vector.memset(box["av"][:], 0.0)
                            av = box["av"]
                            for i in range(2):
                                kt = 2 * kp + i
                                for qi in range(NQ):
                                    nc.tensor.matmul(
                                        av[:, qi, :],
                                        es[kp][:, i * QS + qi * 128:
                                               i * QS + (qi + 1) * 128],
                                        v16[kt][:, h, :],
                                        start=False, stop=(kt == KTILES - 1),
                                        skip_group_check=True,
                                    )
                            return 584
                        return f

                    for kp in range(8):
                        units.append((584, mk_av(kp)))

                    def f_fin():
                        av = box["av"]
                        rec = s2.tile([128, NQ], F32, tag="rec", bufs=2,
                                      name=f"rec{h}_{jq}")
                        nc.vector.reciprocal(rec[:], av[:, :, HD])
                        o_n = s2.tile([128, NQ, HD], F16, tag="on", bufs=2,
                                      name=f"on{h}_{jq}")
                        for qi in range(NQ):
                            nc.vector.tensor_scalar_mul(
                                o_n[:, qi, :], av[:, qi, 0:HD], rec[:, qi:qi + 1]
                            )
                        nxt = ps2.tile([128, NQ, HD + 1], F32, tag="av",
                                       bufs=1, name=f"avn{h}_{jq}")
                        nc.vector.memset(nxt[:], 0.0)
                        av_ready.append(nxt)
                        pt = ps2.tile([HD, NQ, 128], F16, tag="pt", bufs=1,
                                      name=f"pt{h}_{jq}")
                        for qi in range(NQ):
                            nc.tensor.transpose(pt[:, qi, :], o_n[:, qi, :],
                                                ident[:])
                        nc.vector.tensor_copy(
                            o16[h][0:HD, jq * QS:(jq + 1) * QS], pt[:]
                        )
                        return 512
                    units.append((512, f_fin))
                    return units

                ab_units = [mk_proj_ab(ct, NQ - 1) for ct in range(CK)]
                prev = None
                for jq in range(NQ):
                    for h in range(HPG):
                        if prev is not None:
                            fillers.extend(tail_units(*prev))
                            if prev[0] == 3 and prev[1] < NQ - 1:
                                for ct in range(CK):
                                    fillers.extend(mk_proj_filler(ct, prev[1]))
                            if prev[0] == 1 and prev[1] == NQ - 1:
                                fillers.extend(u[0] for u in ab_units)
                        es = emit_scores(h, jq, first=(jq == 0 and h == 0))
                        prev = (h, jq, es)
                fillers.extend(tail_units(*prev))
                fillers.extend(u[1] for u in ab_units)
                drain(10 ** 9)
            _s2cm.__exit__(None, None, None)

    bass_rust.generate_event_semaphores(nc)
    return nc


_NC = None


def _get_nc():
    global _NC
    if _NC is None:
        _NC = build_nc()
    return _NC


def kernel(x, Wqkv, Wproj, bproj, T, H, W):
    x = np.asarray(x, dtype=np.float32)
    Wqkv = np.asarray(Wqkv, dtype=np.float32)
    Wproj = np.asarray(Wproj, dtype=np.float32)
    bproj = np.asarray(bproj, dtype=np.float32)
    assert x.shape == (B, N, C) and Wqkv.shape == (C, 3 * C)
    assert (int(T), int(H), int(W)) == (GT, GH, GW)

    cos96, sin96 = _cos_sin_96()
    nc = _get_nc()

    in_maps = []
    for core in range(NCORES):
        b, g = divmod(core, HPG)
        heads = [HPG * g + i for i in range(HPG)]
        qe = [h * HD + 2 * j for h in heads for j in range(24)]
        qo = [h * HD + 2 * j + 1 for h in heads for j in range(24)]
        qp = [h * HD + 48 + j for h in heads for j in range(24)]
        ke = [C + i for i in qe]
        ko = [C + i for i in qo]
        kp = [C + i for i in qp]
        vcols = [2 * C + h * HD + d for h in heads for d in range(HD)]
        cols = (qe + qp[0:32] + qo + qp[32:64] + ke + qp[64:96]
                + ko + kp[0:32] + kp[32:96] + vcols)
        wqk_c = Wqkv[:, cols].astype(np.float16)
        bias_row = bproj if g == 0 else np.zeros_like(bproj)
        # wp packed [73, HPG*C]: head h columns [h*C:(h+1)*C], row 72 = bias
        # (only meaningful for h==3, whose o16 carries the ones row)
        wp_c = np.zeros((HD + 1, HPG * C), dtype=np.float32)
        for i, h in enumerate(heads):
            wp_c[0:HD, i * C:(i + 1) * C] = Wproj[h * HD:(h + 1) * HD, :]
        wp_c[HD, 3 * C:4 * C] = bias_row
        in_maps.append({
            "xT": np.ascontiguousarray(x[b].T).astype(np.float16),
            "wqk": np.ascontiguousarray(wqk_c),
            "wp": wp_c.astype(np.float16),
            "cosd": cos96.astype(np.float16),
            "sind": sin96.astype(np.float16),
        })

    res = run_bass_kernel_spmd(nc, in_maps, core_ids=list(range(NCORES)))
    out = np.zeros((B, N, C), dtype=np.float32)
    for core in range(NCORES):
        b = core // HPG
        out[b] += res.results[core]["outT"].T
    return out


# revision 45
# speedup vs baseline: 1.0134x; 1.0024x over previous
"""Trainium2 Bass kernel for nn_Attention_79224966742132.

Dense transformer attention block: QKV projection + axial RoPE + SDPA +
output projection, for x (2, 2048, 1152), 16 heads of dim 72.

Sharding (8 cores): data-parallel over batch (2) x tensor-parallel over
head groups (4 heads/core). Each core computes QKV for its 4 heads from
the full x[b], applies RoPE, runs attention, and produces a partial
output projection (row-parallel Wproj); the host sums the 4 partials per
batch element. The projection bias rides as an extra contraction row on
the g==0 core of each batch.

v3 design notes (against the TimelineSim cost model):
- All phase-1 matmuls in fp16 (1 cycle/row at any moving size); x, Wqkv,
  Wv are quantized to fp16 on the host (~1e-3 rel err, gate is 2e-2).
- QK projection packed into 5 stationary blocks of <=128 columns
  (4x128 + 64) instead of 6x96: pass-dims fill the block remainders.
- Attention-value matmul restructured: exp-weights tile [128kt, 128qt]
  is the STATIONARY operand, v [128, 73] fp16 the moving one -> 73
  cycles per k-tile instead of 512 (output lands as [qtok, hd]; a cheap
  PE transpose brings it back to [hd, qtok] for the projection).
- Softmax denominator = ones column appended to v; reciprocal + scale on
  DVE in the [qtok, hd] layout (per-partition scalar, no broadcasts).
- Projection bias folded into the h3 projection matmul as a 73rd
  contraction row against a ones row in o16[3].
- The exp on ACT (133us) is the phase-2 near-critical path; V for token
  halves 2-3, all projection matmuls, and each chunk's AV/normalize/
  transpose tail are deferred into a filler queue drained between score
  matmuls so PE never idles while ACT catches up.
- Half-0 scores+exps for chunks (h0,jq0)/(h1,jq0) are emitted inside
  phase 1 (hooked between qn3's blocks) so ACT starts ~20us early.
- HWDGE descriptor-gen is a single serialized device (~625ns/DMA) and
  DMA transfers serialize on one modeled DMA_ENGINES device: DMA count
  is minimized (wv merged into the wqk tensor, wp packed into one
  tile); latency-critical repacks go HWDGE, the rest SWDGE (Pool).
- PSUM accumulation groups are zeroed per BANK by matmul start=True, so
  the four AV sub-tile groups sharing one bank are memset once and
  accumulated with start=False.
- GPSIMD cannot touch PSUM and engine APs need 32-aligned partition
  offsets (hardware rules the cost model does not check).
"""
import math
import os
import sys
from collections import deque

# The device path needs the axon/neuron jax platform; if a harness pinned
# JAX_PLATFORMS=cpu (common for running jax references) and jax is not yet
# imported, restore platform auto-detection.
if "jax" not in sys.modules:
    _jp = os.environ.get("JAX_PLATFORMS")
    if _jp and "axon" not in _jp and "neuron" not in _jp:
        del os.environ["JAX_PLATFORMS"]

import numpy as np

import bass_rust
import concourse.bass as bass
import concourse.mybir as mybir
import concourse.tile as tile
from concourse.bass_utils import run_bass_kernel_spmd
from concourse.masks import make_identity

F32 = mybir.dt.float32
F16 = mybir.dt.float16
AF = mybir.ActivationFunctionType
ALU = mybir.AluOpType

B = 2
N = 2048          # tokens = T*H*W = 8*16*16
C = 1152
NH = 16
HD = 72
HPG = 4           # heads per core
NCORES = 8
GT, GH, GW = 8, 16, 16
SCALE = 1.0 / math.sqrt(HD)

NQ = 4            # q-chunks (512 tokens each) and qt-subtiles per chunk
QS = N // NQ      # 512
KTILES = N // 128  # 16
CK = C // 128      # 9 contraction chunks
HS = N // 2        # RoPE-output/repack half granularity


def _axis_freqs(n: int) -> np.ndarray:
    base = np.linspace(1.0, 128.0, 8, dtype=np.float64) * np.pi   # MAX_FREQ/2
    pos = np.linspace(-1.0, 1.0, n, dtype=np.float64)
    return pos[:, None] * base[None, :]                            # (n, 8)


def _cos_sin_96():
    """cos/sin of the 24 pair frequencies per token, tiled x4 -> (96, N)."""
    f = np.zeros((GT, GH, GW, 24), dtype=np.float64)
    f[..., 0:8] = _axis_freqs(GT)[:, None, None, :]
    f[..., 8:16] = _axis_freqs(GH)[None, :, None, :]
    f[..., 16:24] = _axis_freqs(GW)[None, None, :, :]
    f = f.reshape(N, 24)
    cos24 = np.ascontiguousarray(np.cos(f).astype(np.float32).T)   # (24, N)
    sin24 = np.ascontiguousarray(np.sin(f).astype(np.float32).T)
    return np.tile(cos24, (4, 1)), np.tile(sin24, (4, 1))          # (96, N)


def build_nc() -> bass.Bass:
    nc = bass.Bass()
    xT = nc.dram_tensor("xT", [C, N], F16, kind="ExternalInput")
    wqk = nc.dram_tensor("wqk", [C, 576 + HPG * HD], F16, kind="ExternalInput")
    wp = nc.dram_tensor("wp", [HD + 1, HPG * C], F16, kind="ExternalInput")
    cosd = nc.dram_tensor("cosd", [96, N], F16, kind="ExternalInput")
    sind = nc.dram_tensor("sind", [96, N], F16, kind="ExternalInput")
    outT = nc.dram_tensor("outT", [C, N], F32, kind="ExternalOutput")

    with tile.TileContext(nc) as tc:
        with tc.tile_pool(name="persist", bufs=1) as pp:
            x16 = [pp.tile([128, N], F16, name=f"x16_{k}") for k in range(CK)]
            qt16 = pp.tile([HD, HPG * N], F16, name="qt16")
            kt16 = pp.tile([HD, HPG * N], F16, name="kt16")
            v16 = [pp.tile([128, HPG, HD + 1], F16, name=f"v16_{i}")
                   for i in range(KTILES)]
            o16 = [pp.tile([HD + (1 if h == 3 else 0), N], F16, name=f"o16_{h}")
                   for h in range(HPG)]
            wqk_t = [pp.tile([128, 576 + HPG * HD], F16, name=f"wqk{k}")
                     for k in range(CK)]
            wp4 = pp.tile([HD + 1, HPG, C], F16, name="wp4")
            cos_t = pp.tile([96, N], F16, name="cos_t")
            sin_t = pp.tile([96, N], F16, name="sin_t")
            ident = pp.tile([128, 128], F16, name="ident")

            ones_row = pp.tile([1, N], F16, name="ones_row")
            make_identity(nc, ident[:])
            for i in range(KTILES):
                nc.vector.memset(v16[i][:, :, HD], 1.0)
            nc.vector.memset(ones_row[:], 1.0)
            # engine writes need 32-aligned partition offsets; DMA does not
            nc.sync.dma_start(o16[3][HD:HD + 1, :], ones_row[:])

            # bulk loads alternate between the two HWDGE issuers (SP + ACT,
            # 16 queues each) so transfers run in parallel and neither
            # sequencer serializes the load phase
            _eng = [nc.sync, nc.scalar]
            _ei = [0]

            def dma(out, in_):
                _eng[_ei[0] & 1].dma_start(out, in_)
                _ei[0] += 1

            for k in range(CK):
                dma(wqk_t[k][:], wqk[k * 128:(k + 1) * 128, :])
                dma(x16[k][:, 0:HS], xT[k * 128:(k + 1) * 128, 0:HS])
            dma(cos_t[:], cosd[:, :])
            dma(sin_t[:], sind[:, :])
            for k in range(CK):
                dma(x16[k][:, HS:N], xT[k * 128:(k + 1) * 128, HS:N])
            dma(wp4[:], wp[:].rearrange("p (h c) -> p h c", h=HPG))

            # ---------------- emit helpers ----------------

            def emit_qkrope(ps_pool, sb_pool, qn, halves, hook=None):
                """5-block QK matmuls + RoPE for one token quarter, 4 heads.

                Column blocks (stationary, host-packed):
                  B0 = Qe(96) + Qp[0:32]     B1 = Qo(96) + Qp[32:64]
                  B2 = Ke(96) + Qp[64:96]    B3 = Ko(96) + Kp[0:32]
                  B4 = Kp[32:96]
                where e/o/p = rotary-even/odd/pass dims, head-major.
                RoPE for Q is emitted right after B1 (and K after B3) so the
                DVE chain starts early and single-buffered PSUM blocks never
                stall the next quarter.
                """
                ts0 = qn * QS
                hn, sub = divmod(qn, 2)
                sl = slice(sub * QS, (sub + 1) * QS)
                erq, orq, prq, erk, ork, prk = halves[hn]
                cosq = cos_t[:, ts0:ts0 + QS]
                sinq = sin_t[:, ts0:ts0 + QS]

                def mm_block(m):
                    w = 64 if m == 4 else 128
                    blk = ps_pool.tile([w, QS], F32, tag=f"qk{m}", bufs=1,
                                       name=f"qk{qn}_{m}")
                    for k in range(CK):
                        nc.tensor.matmul(
                            blk[:],
                            wqk_t[k][:, 128 * m:128 * m + w],
                            x16[k][:, ts0:ts0 + QS],
                            start=(k == 0), stop=(k == CK - 1),
                        )
                    return blk

                def rope(e_blk, o_blk, er, orr):
                    t1 = sb_pool.tile([96, QS], F16, tag="rtA", bufs=1,
                                      name=f"t1_{qn}")
                    t2 = sb_pool.tile([96, QS], F16, tag="rtB", bufs=1,
                                      name=f"t2_{qn}")
                    nc.vector.tensor_tensor(t1[:], e_blk[0:96, :], cosq, ALU.mult)
                    nc.vector.tensor_tensor(t2[:], o_blk[0:96, :], sinq, ALU.mult)
                    nc.vector.tensor_tensor(er[:, sl], t1[:], t2[:], ALU.subtract)
                    t3 = sb_pool.tile([96, QS], F16, tag="rtA", bufs=1,
                                      name=f"t3_{qn}")
                    t4 = sb_pool.tile([96, QS], F16, tag="rtB", bufs=1,
                                      name=f"t4_{qn}")
                    nc.vector.tensor_tensor(t3[:], o_blk[0:96, :], cosq, ALU.mult)
                    nc.vector.tensor_tensor(t4[:], e_blk[0:96, :], sinq, ALU.mult)
                    nc.vector.tensor_tensor(orr[:, sl], t3[:], t4[:], ALU.add)

                if qn == 3:
                    # K first: the half-1 kt repack gates phase 2
                    B2 = mm_block(2)
                    if hook: hook()
                    B3 = mm_block(3)
                    rope(B2, B3, erk, ork)
                    if hook: hook()
                    B0 = mm_block(0)
                    if hook: hook()
                    B1 = mm_block(1)
                    rope(B0, B1, erq, orq)
                    if hook: hook()
                else:
                    B0 = mm_block(0)
                    B1 = mm_block(1)
                    rope(B0, B1, erq, orq)
                    B2 = mm_block(2)
                    B3 = mm_block(3)
                    rope(B2, B3, erk, ork)
                B4 = mm_block(4)
                # pass dims: Qp spread over B0/B1/B2 remainders, Kp over B3/B4.
                # The last quarter's copies go to the (idle) ACT engine so the
                # PSUM banks free up fast for phase 2.
                nc.scalar.copy(prq[0:32, sl], B0[96:128, :])
                nc.scalar.copy(prq[32:64, sl], B1[96:128, :])
                nc.scalar.copy(prq[64:96, sl], B2[96:128, :])
                nc.scalar.copy(prk[0:32, sl], B3[96:128, :])
                nc.scalar.copy(prk[32:64, sl], B4[0:32, :])
                nc.scalar.copy(prk[64:96, sl], B4[32:64, :])

            def emit_repack(hn, halves, part="both", quarter=None):
                """DMA the rotated halves into per-head [72, N] q/k tiles.

                Per-head dim order: [0:24] even-rotated, [24:48] odd-rotated,
                [48:72] pass -- same permutation for q and k, so scores are
                unchanged. Issues alternate between the two HWDGE engines.
                """
                erq, orq, prq, erk, ork, prk = halves[hn]
                if quarter is None:
                    cs, sz = slice(0, HS), HS
                else:
                    cs, sz = slice(quarter * QS, (quarter + 1) * QS), QS
                hs0 = hn * HS + (0 if quarter is None else quarter * QS)
                qdma = dma if hn == 0 else nc.gpsimd.dma_start
                if part in ("both", "kt"):
                    for h in range(HPG):
                        d0 = h * N + hs0
                        r = slice(24 * h, 24 * h + 24)
                        if hn == 1:
                            # ACT.SEQ is running partA exps when these issue;
                            # keep the critical kt repack off it entirely
                            eng = nc.gpsimd.dma_start if h % 2 else nc.sync.dma_start
                        else:
                            eng = dma
                        eng(kt16[0:24, d0:d0 + sz], erk[r, cs])
                        eng(kt16[24:48, d0:d0 + sz], ork[r, cs])
                        eng(kt16[48:72, d0:d0 + sz], prk[r, cs])
                if part in ("both", "qt"):
                    for h in range(HPG):
                        d0 = h * N + hs0
                        r = slice(24 * h, 24 * h + 24)
                        qdma(qt16[0:24, d0:d0 + sz], erq[r, cs])
                        qdma(qt16[24:48, d0:d0 + sz], orq[r, cs])
                        qdma(qt16[48:72, d0:d0 + sz], prq[r, cs])

            def emit_v_tt(qn, tt, ps_pool, ks=range(CK), box=None):
                """V for all 4 heads, one 128-token tile, x-stationary.
                ks selects the contraction slice so fillers can split the
                accumulation into small units (box carries the psum tile)."""
                ts0 = qn * QS
                if box is None:
                    box = {}
                if "vp" not in box:
                    box["vp"] = ps_pool.tile([128, QS], F32, tag="op", bufs=2,
                                             name=f"vps{qn}_{tt}")
                vp = box["vp"]
                for k in ks:
                    nc.tensor.matmul(
                        vp[:, 0:HPG * HD],
                        x16[k][:, ts0 + tt * 128:ts0 + (tt + 1) * 128],
                        wqk_t[k][:, 576:576 + HPG * HD],
                        start=(k == 0), stop=(k == CK - 1),
                    )
                if ks[-1] == CK - 1:
                    cp = nc.scalar.copy if qn < 2 else nc.vector.tensor_copy
                    cp(
                        v16[qn * 4 + tt][:, :, 0:HD],
                        vp[:, 0:HPG * HD].rearrange("p (h d) -> p h d", h=HPG),
                    )

            def emit_proj(ct, jq, ps_pool, sb_pool):
                op = ps_pool.tile([128, QS], F32, tag="op", bufs=2,
                                  name=f"op{ct}_{jq}")
                for i in range(HPG):
                    hd2 = HD + 1 if i == 3 else HD
                    nc.tensor.matmul(
                        op[:], wp4[0:hd2, i, ct * 128:(ct + 1) * 128],
                        o16[i][:, jq * QS:(jq + 1) * QS],
                        start=(i == 0), stop=(i == HPG - 1),
                    )
                osb = sb_pool.tile([128, QS], F32, tag="osb", bufs=3,
                                   name=f"osb{ct}_{jq}")
                # copies alternate DVE/Pool; out-DMA issues from SP (the ACT
                # sequencer is saturated with exps in phase 2)
                if ct % 2 == 0:
                    nc.gpsimd.tensor_copy(osb[:], op[:])
                else:
                    nc.vector.tensor_copy(osb[:], op[:])
                nc.sync.dma_start(
                    outT[ct * 128:(ct + 1) * 128, jq * QS:(jq + 1) * QS], osb[:]
                )

            # ================= phase 1: QKV + RoPE + repack =================
            _s2cm = tc.tile_pool(name="s2", bufs=1)
            s2 = _s2cm.__enter__()
            early_es = {}

            def emit_partA_kp(h, kp):
                """Scores+exp for one kp of chunk (h, jq=0), emitted inside
                phase 1 once the half-0 repack is in flight. Uses two [128,QS]
                PSUM tiles from the shared 'op' tag and f512 exps so no extra
                banks are needed."""
                hb = h * N
                if True:
                    sts = []
                    for i in range(2):
                        kt = 2 * kp + i
                        stx = ps1.tile([128, QS], F32, tag="op", bufs=2,
                                       name=f"stE{h}_{kp}_{i}")
                        nc.tensor.matmul(
                            stx[:],
                            kt16[:, hb + kt * 128:hb + (kt + 1) * 128],
                            qt16[:, hb:hb + QS],
                            start=True, stop=True,
                        )
                        sts.append(stx)
                    e = s2.tile([128, 2 * QS], F16, tag="e", bufs=23,
                                name=f"eE{h}_{kp}")
                    for i in range(2):
                        nc.scalar.activation(e[:, i * QS:(i + 1) * QS],
                                             sts[i][:], AF.Exp, scale=SCALE)
                    early_es.setdefault(h, []).append(e)

            def emit_partA(h, ps_pool, between=None):
                for kp in range(4):
                    if between is not None:
                        between()
                    emit_partA_kp(h, kp)

            with (
                tc.tile_pool(name="s1", bufs=1) as s1,
                tc.tile_pool(name="ps1", bufs=1, space="PSUM") as ps1,
            ):
                halves = [
                    tuple(
                        s1.tile([96, HS], F16, tag=f"{nm}", bufs=1,
                                name=f"{nm}_{hn}")
                        for nm in ("erq", "orq", "prq", "erk", "ork", "prk")
                    )
                    for hn in range(2)
                ]
                kp_ctr = [0]

                def hook():
                    if kp_ctr[0] < 4:
                        emit_partA_kp(0, kp_ctr[0])
                        kp_ctr[0] += 1

                for qn in range(4):
                    emit_qkrope(ps1, s1, qn, halves,
                                hook=hook if qn == 3 else None)
                    if qn == 1:
                        for tt in range(4):
                            emit_v_tt(0, tt, ps1)
                        emit_repack(0, halves)
                    if qn == 2:
                        for tt in range(4):
                            emit_v_tt(1, tt, ps1)
                        emit_repack(1, halves, part="kt", quarter=0)
                    if qn == 3:
                        emit_repack(1, halves, part="kt", quarter=1)
                        emit_partA(1, ps1)
                        emit_repack(1, halves, part="qt")

            # ================= phase 2: attention + projection ===============
            with tc.tile_pool(name="ps2", bufs=1, space="PSUM") as ps2:
                fillers = deque()
                for qn in (2, 3):
                    for tt in range(4):
                        vbox = {}
                        for ks in (range(0, 3), range(3, 6), range(6, CK)):
                            fillers.append((288 * len(ks),
                                            lambda qn=qn, tt=tt, ks=ks, vbox=vbox:
                                            emit_v_tt(qn, tt, ps2, ks, vbox)))

                def mk_proj_filler(ct, jq):
                    # two units: heads 0-1, then heads 2-3 + copy + store
                    pbox = {}

                    def a():
                        pbox["op"] = ps2.tile([128, QS], F32, tag="op", bufs=2,
                                              name=f"op{ct}_{jq}")
                        for i in (0, 1):
                            nc.tensor.matmul(
                                pbox["op"][:],
                                wp4[0:HD, i, ct * 128:(ct + 1) * 128],
                                o16[i][:, jq * QS:(jq + 1) * QS],
                                start=(i == 0), stop=False,
                            )
                        return 1024

                    def b():
                        op = pbox["op"]
                        for i in (2, 3):
                            hd2 = HD + 1 if i == 3 else HD
                            nc.tensor.matmul(
                                op[:], wp4[0:hd2, i, ct * 128:(ct + 1) * 128],
                                o16[i][:, jq * QS:(jq + 1) * QS],
                                start=False, stop=(i == 3),
                            )
                        osb = s2.tile([128, QS], F32, tag="osb", bufs=3,
                                      name=f"osb{ct}_{jq}")
                        nc.vector.tensor_copy(osb[:], op[:])
                        nc.sync.dma_start(
                            outT[ct * 128:(ct + 1) * 128,
                                 jq * QS:(jq + 1) * QS], osb[:]
                        )
                        return 1664
                    return [(1024, a), (1664, b)]

                def mk_proj_ab(ct, jq):
                    pbox = {}

                    def a():
                        op = ps2.tile([128, QS], F32, tag="op", bufs=2,
                                      name=f"opA{ct}_{jq}")
                        for i in (0, 1):
                            nc.tensor.matmul(
                                op[:], wp4[0:HD, i, ct * 128:(ct + 1) * 128],
                                o16[i][:, jq * QS:(jq + 1) * QS],
                                start=(i == 0), stop=(i == 1),
                            )
                        park = s2.tile([128, QS], F16, tag="park", bufs=9,
                                       name=f"park{ct}")
                        nc.vector.tensor_copy(park[:], op[:])
                        pbox["park"] = park
                        return 1024

                    def b():
                        op = ps2.tile([128, QS], F32, tag="op", bufs=2,
                                      name=f"opB{ct}_{jq}")
                        for i in (2, 3):
                            hd2 = HD + 1 if i == 3 else HD
                            nc.tensor.matmul(
                                op[:], wp4[0:hd2, i, ct * 128:(ct + 1) * 128],
                                o16[i][:, jq * QS:(jq + 1) * QS],
                                start=(i == 2), stop=(i == 3),
                            )
                        osb = s2.tile([128, QS], F32, tag="osb", bufs=3,
                                      name=f"osb{ct}_{jq}")
                        nc.vector.tensor_tensor(osb[:], op[:],
                                                pbox["park"][:], ALU.add)
                        nc.sync.dma_start(
                            outT[ct * 128:(ct + 1) * 128,
                                 jq * QS:(jq + 1) * QS], osb[:]
                        )
                        return 1664
                    return (1024, a), (1664, b)

                def drain(budget):
                    while fillers and budget > 0:
                        cost, fn = fillers.popleft()
                        fn()
                        budget -= cost

                def emit_scores(h, jq, first=False):
                    hb = h * N
                    es = []
                    kps = range(8)
                    if jq == 0 and h in early_es:
                        es = list(early_es[h])
                        kps = range(4, 8)
                    for kp in kps:
                        st = ps2.tile([128, 2 * QS], F32, tag="st", bufs=2,
                                      name=f"st{h}_{jq}_{kp}")
                        for i in range(2):
                            kt = 2 * kp + i
                            nc.tensor.matmul(
                                st[:, i * QS:(i + 1) * QS],
                                kt16[:, hb + kt * 128:hb + (kt + 1) * 128],
                                qt16[:, hb + jq * QS:hb + (jq + 1) * QS],
                                start=True, stop=True,
                            )
                        e = s2.tile([128, 2 * QS], F16, tag="e", bufs=23,
                                    name=f"e{h}_{jq}_{kp}")
                        nc.scalar.activation(e[:], st[:], AF.Exp, scale=SCALE)
                        es.append(e)
                        if kps[0] == 4:
                            drain(2100)
                        else:
                            drain(0 if kp < 2 else 2100)
                    return es

                av_ready = deque()

                def tail_units(h, jq, es):
                    """AV + normalize + transpose for a finished chunk, as filler units."""
                    units = []
                    box = {}

                    def mk_av(kp):
                        def f():
                            if kp == 0:
                                if av_ready:
                                    box["av"] = av_ready.popleft()
                                else:
                                    box["av"] = ps2.tile(
                                        [128, NQ, HD + 1], F32, tag="av",
                                        bufs=1, name=f"av{h}_{jq}")
                                    # 4 accumulation regions share one PSUM
                                    # bank; a start=True zeroes the whole
                                    # bank, so zero once, accumulate with
                                    # start=False
                                    nc.vector.memset(box["av"][:], 0.0)
                            av = box["av"]
                            for i in range(2):
                                kt = 2 * kp + i
                                for qi in range(NQ):
                                    nc.tensor.matmul(
                                        av[:, qi, :],
                                        es[kp][:, i * QS + qi * 128:
                                               i * QS + (qi + 1) * 128],
                                        v16[kt][:, h, :],
                                        start=False, stop=(kt == KTILES - 1),
                                        skip_group_check=True,
                                    )
                            return 584
                        return f

                    for kp in range(8):
                        units.append((584, mk_av(kp)))

                    def f_fin():
                        av = box["av"]
                        rec = s2.tile([128, NQ], F32, tag="rec", bufs=2,
                                      name=f"rec{h}_{jq}")
                        nc.vector.reciprocal(rec[:], av[:, :, HD])
                        o_n = s2.tile([128, NQ, HD], F16, tag="on", bufs=2,
                                      name=f"on{h}_{jq}")
                        for qi in range(NQ):
                            nc.vector.tensor_scalar_mul(
                                o_n[:, qi, :], av[:, qi, 0:HD], rec[:, qi:qi + 1]
                            )
                        nxt = ps2.tile([128, NQ, HD + 1], F32, tag="av",
                                       bufs=1, name=f"avn{h}_{jq}")
                        nc.vector.memset(nxt[:], 0.0)
                        av_ready.append(nxt)
                        pt = ps2.tile([HD, NQ, 128], F16, tag="pt", bufs=1,
                                      name=f"pt{h}_{jq}")
                        for qi in range(NQ):
                            nc.tensor.transpose(pt[:, qi, :], o_n[:, qi, :],
                                                ident[:])
                        nc.vector.tensor_copy(
                            o16[h][0:HD, jq * QS:(jq + 1) * QS], pt[:]
                        )
                        return 512
                    units.append((512, f_fin))
                    return units

                ab_units = [mk_proj_ab(ct, NQ - 1) for ct in range(CK)]
                prev = None
                for jq in range(NQ):
                    for h in range(HPG):
                        if prev is not None:
                            fillers.extend(tail_units(*prev))
                            if prev[0] == 3 and prev[1] < NQ - 1:
                                for ct in range(CK):
                                    fillers.extend(mk_proj_filler(ct, prev[1]))
                            if prev[0] == 1 and prev[1] == NQ - 1:
                                fillers.extend(u[0] for u in ab_units)
                        es = emit_scores(h, jq, first=(jq == 0 and h == 0))
                        prev = (h, jq, es)
                fillers.extend(tail_units(*prev))
                fillers.extend(u[1] for u in ab_units)
                drain(10 ** 9)
            _s2cm.__exit__(None, None, None)

    bass_rust.generate_event_semaphores(nc)
    return nc


_NC = None


def _get_nc():
    global _NC
    if _NC is None:
        _NC = build_nc()
    return _NC


def kernel(x, Wqkv, Wproj, bproj, T, H, W):
    x = np.asarray(x, dtype=np.float32)
    Wqkv = np.asarray(Wqkv, dtype=np.float32)
    Wproj = np.asarray(Wproj, dtype=np.float32)
    bproj = np.asarray(bproj, dtype=np.float32)
    assert x.shape == (B, N, C) and Wqkv.shape == (C, 3 * C)
    assert (int(T), int(H), int(W)) == (GT, GH, GW)

    cos96, sin96 = _cos_sin_96()
    nc = _get_nc()

    in_maps = []
    for core in range(NCORES):
        b, g = divmod(core, HPG)
        heads = [HPG * g + i for i in range(HPG)]
        qe = [h * HD + 2 * j for h in heads for j in range(24)]
        qo = [h * HD + 2 * j + 1 for h in heads for j in range(24)]
        qp = [h * HD + 48 + j for h in heads for j in range(24)]
        ke = [C + i for i in qe]
        ko = [C + i for i in qo]
        kp = [C + i for i in qp]
        vcols = [2 * C + h * HD + d for h in heads for d in range(HD)]
        cols = (qe + qp[0:32] + qo + qp[32:64] + ke + qp[64:96]
                + ko + kp[0:32] + kp[32:96] + vcols)
        wqk_c = Wqkv[:, cols].astype(np.float16)
        bias_row = bproj if g == 0 else np.zeros_like(bproj)
        # wp packed [73, HPG*C]: head h columns [h*C:(h+1)*C], row 72 = bias
        # (only meaningful for h==3, whose o16 carries the ones row)
        wp_c = np.zeros((HD + 1, HPG * C), dtype=np.float32)
        for i, h in enumerate(heads):
            wp_c[0:HD, i * C:(i + 1) * C] = Wproj[h * HD:(h + 1) * HD, :]
        wp_c[HD, 3 * C:4 * C] = bias_row
        in_maps.append({
            "xT": np.ascontiguousarray(x[b].T).astype(np.float16),
            "wqk": np.ascontiguousarray(wqk_c),
            "wp": wp_c.astype(np.float16),
            "cosd": cos96.astype(np.float16),
            "sind": sin96.astype(np.float16),
        })

    res = run_bass_kernel_spmd(nc, in_maps, core_ids=list(range(NCORES)))
    out = np.zeros((B, N, C), dtype=np.float32)
    for core in range(NCORES):
        b = core // HPG
        out[b] += res.results[core]["outT"].T
    return out


# revision 46
# speedup vs baseline: 1.0146x; 1.0012x over previous
"""Trainium2 Bass kernel for nn_Attention_79224966742132.

Dense transformer attention block: QKV projection + axial RoPE + SDPA +
output projection, for x (2, 2048, 1152), 16 heads of dim 72.

Sharding (8 cores): data-parallel over batch (2) x tensor-parallel over
head groups (4 heads/core). Each core computes QKV for its 4 heads from
the full x[b], applies RoPE, runs attention, and produces a partial
output projection (row-parallel Wproj); the host sums the 4 partials per
batch element. The projection bias rides as an extra contraction row on
the g==0 core of each batch.

v3 design notes (against the TimelineSim cost model):
- All phase-1 matmuls in fp16 (1 cycle/row at any moving size); x, Wqkv,
  Wv are quantized to fp16 on the host (~1e-3 rel err, gate is 2e-2).
- QK projection packed into 5 stationary blocks of <=128 columns
  (4x128 + 64) instead of 6x96: pass-dims fill the block remainders.
- Attention-value matmul restructured: exp-weights tile [128kt, 128qt]
  is the STATIONARY operand, v [128, 73] fp16 the moving one -> 73
  cycles per k-tile instead of 512 (output lands as [qtok, hd]; a cheap
  PE transpose brings it back to [hd, qtok] for the projection).
- Softmax denominator = ones column appended to v; reciprocal + scale on
  DVE in the [qtok, hd] layout (per-partition scalar, no broadcasts).
- Projection bias folded into the h3 projection matmul as a 73rd
  contraction row against a ones row in o16[3].
- The exp on ACT (133us) is the phase-2 near-critical path; V for token
  halves 2-3, all projection matmuls, and each chunk's AV/normalize/
  transpose tail are deferred into a filler queue drained between score
  matmuls so PE never idles while ACT catches up.
- Half-0 scores+exps for chunks (h0,jq0)/(h1,jq0) are emitted inside
  phase 1 (hooked between qn3's blocks) so ACT starts ~20us early.
- HWDGE descriptor-gen is a single serialized device (~625ns/DMA) and
  DMA transfers serialize on one modeled DMA_ENGINES device: DMA count
  is minimized (wv merged into the wqk tensor, wp packed into one
  tile); latency-critical repacks go HWDGE, the rest SWDGE (Pool).
- PSUM accumulation groups are zeroed per BANK by matmul start=True, so
  the four AV sub-tile groups sharing one bank are memset once and
  accumulated with start=False.
- GPSIMD cannot touch PSUM and engine APs need 32-aligned partition
  offsets (hardware rules the cost model does not check).
"""
import math
import os
import sys
from collections import deque

# The device path needs the axon/neuron jax platform; if a harness pinned
# JAX_PLATFORMS=cpu (common for running jax references) and jax is not yet
# imported, restore platform auto-detection.
if "jax" not in sys.modules:
    _jp = os.environ.get("JAX_PLATFORMS")
    if _jp and "axon" not in _jp and "neuron" not in _jp:
        del os.environ["JAX_PLATFORMS"]

import numpy as np

import bass_rust
import concourse.bass as bass
import concourse.mybir as mybir
import concourse.tile as tile
from concourse.bass_utils import run_bass_kernel_spmd
from concourse.masks import make_identity

F32 = mybir.dt.float32
F16 = mybir.dt.float16
AF = mybir.ActivationFunctionType
ALU = mybir.AluOpType

B = 2
N = 2048          # tokens = T*H*W = 8*16*16
C = 1152
NH = 16
HD = 72
HPG = 4           # heads per core
NCORES = 8
GT, GH, GW = 8, 16, 16
SCALE = 1.0 / math.sqrt(HD)

NQ = 4            # q-chunks (512 tokens each) and qt-subtiles per chunk
QS = N // NQ      # 512
KTILES = N // 128  # 16
CK = C // 128      # 9 contraction chunks
HS = N // 2        # RoPE-output/repack half granularity


def _axis_freqs(n: int) -> np.ndarray:
    base = np.linspace(1.0, 128.0, 8, dtype=np.float64) * np.pi   # MAX_FREQ/2
    pos = np.linspace(-1.0, 1.0, n, dtype=np.float64)
    return pos[:, None] * base[None, :]                            # (n, 8)


def _cos_sin_96():
    """cos/sin of the 24 pair frequencies per token, tiled x4 -> (96, N)."""
    f = np.zeros((GT, GH, GW, 24), dtype=np.float64)
    f[..., 0:8] = _axis_freqs(GT)[:, None, None, :]
    f[..., 8:16] = _axis_freqs(GH)[None, :, None, :]
    f[..., 16:24] = _axis_freqs(GW)[None, None, :, :]
    f = f.reshape(N, 24)
    cos24 = np.ascontiguousarray(np.cos(f).astype(np.float32).T)   # (24, N)
    sin24 = np.ascontiguousarray(np.sin(f).astype(np.float32).T)
    return np.tile(cos24, (4, 1)), np.tile(sin24, (4, 1))          # (96, N)


def build_nc() -> bass.Bass:
    nc = bass.Bass()
    xT = nc.dram_tensor("xT", [C, N], F16, kind="ExternalInput")
    wqk = nc.dram_tensor("wqk", [C, 576 + HPG * HD], F16, kind="ExternalInput")
    wp = nc.dram_tensor("wp", [HD + 1, HPG * C], F16, kind="ExternalInput")
    cosd = nc.dram_tensor("cosd", [96, N], F16, kind="ExternalInput")
    sind = nc.dram_tensor("sind", [96, N], F16, kind="ExternalInput")
    outT = nc.dram_tensor("outT", [C, N], F32, kind="ExternalOutput")

    with tile.TileContext(nc) as tc:
        with tc.tile_pool(name="persist", bufs=1) as pp:
            x16 = [pp.tile([128, N], F16, name=f"x16_{k}") for k in range(CK)]
            qt16 = pp.tile([HD, HPG * N], F16, name="qt16")
            kt16 = pp.tile([HD, HPG * N], F16, name="kt16")
            v16 = [pp.tile([128, HPG, HD + 1], F16, name=f"v16_{i}")
                   for i in range(KTILES)]
            o16 = [pp.tile([HD + (1 if h == 3 else 0), N], F16, name=f"o16_{h}")
                   for h in range(HPG)]
            wqk_t = [pp.tile([128, 576 + HPG * HD], F16, name=f"wqk{k}")
                     for k in range(CK)]
            wp4 = pp.tile([HD + 1, HPG, C], F16, name="wp4")
            cos_t = pp.tile([96, N], F16, name="cos_t")
            sin_t = pp.tile([96, N], F16, name="sin_t")
            ident = pp.tile([128, 128], F16, name="ident")

            ones_row = pp.tile([1, N], F16, name="ones_row")
            make_identity(nc, ident[:])
            for i in range(KTILES):
                nc.vector.memset(v16[i][:, :, HD], 1.0)
            nc.vector.memset(ones_row[:], 1.0)
            # engine writes need 32-aligned partition offsets; DMA does not
            nc.sync.dma_start(o16[3][HD:HD + 1, :], ones_row[:])

            # bulk loads alternate between the two HWDGE issuers (SP + ACT,
            # 16 queues each) so transfers run in parallel and neither
            # sequencer serializes the load phase
            _eng = [nc.sync, nc.scalar]
            _ei = [0]

            def dma(out, in_):
                _eng[_ei[0] & 1].dma_start(out, in_)
                _ei[0] += 1

            for k in range(CK):
                dma(wqk_t[k][:], wqk[k * 128:(k + 1) * 128, :])
                dma(x16[k][:, 0:HS], xT[k * 128:(k + 1) * 128, 0:HS])
            dma(cos_t[:], cosd[:, :])
            dma(sin_t[:], sind[:, :])
            for k in range(CK):
                dma(x16[k][:, HS:N], xT[k * 128:(k + 1) * 128, HS:N])
            dma(wp4[:], wp[:].rearrange("p (h c) -> p h c", h=HPG))

            # ---------------- emit helpers ----------------

            def emit_qkrope(ps_pool, sb_pool, qn, halves, hook=None):
                """5-block QK matmuls + RoPE for one token quarter, 4 heads.

                Column blocks (stationary, host-packed):
                  B0 = Qe(96) + Qp[0:32]     B1 = Qo(96) + Qp[32:64]
                  B2 = Ke(96) + Qp[64:96]    B3 = Ko(96) + Kp[0:32]
                  B4 = Kp[32:96]
                where e/o/p = rotary-even/odd/pass dims, head-major.
                RoPE for Q is emitted right after B1 (and K after B3) so the
                DVE chain starts early and single-buffered PSUM blocks never
                stall the next quarter.
                """
                ts0 = qn * QS
                hn, sub = divmod(qn, 2)
                sl = slice(sub * QS, (sub + 1) * QS)
                erq, orq, prq, erk, ork, prk = halves[hn]
                cosq = cos_t[:, ts0:ts0 + QS]
                sinq = sin_t[:, ts0:ts0 + QS]

                def mm_block(m):
                    w = 64 if m == 4 else 128
                    blk = ps_pool.tile([w, QS], F32, tag=f"qk{m}", bufs=1,
                                       name=f"qk{qn}_{m}")
                    for k in range(CK):
                        nc.tensor.matmul(
                            blk[:],
                            wqk_t[k][:, 128 * m:128 * m + w],
                            x16[k][:, ts0:ts0 + QS],
                            start=(k == 0), stop=(k == CK - 1),
                        )
                    return blk

                def rope(e_blk, o_blk, er, orr):
                    t1 = sb_pool.tile([96, QS], F16, tag="rtA", bufs=1,
                                      name=f"t1_{qn}")
                    t2 = sb_pool.tile([96, QS], F16, tag="rtB", bufs=1,
                                      name=f"t2_{qn}")
                    nc.vector.tensor_tensor(t1[:], e_blk[0:96, :], cosq, ALU.mult)
                    nc.vector.tensor_tensor(t2[:], o_blk[0:96, :], sinq, ALU.mult)
                    nc.vector.tensor_tensor(er[:, sl], t1[:], t2[:], ALU.subtract)
                    t3 = sb_pool.tile([96, QS], F16, tag="rtA", bufs=1,
                                      name=f"t3_{qn}")
                    t4 = sb_pool.tile([96, QS], F16, tag="rtB", bufs=1,
                                      name=f"t4_{qn}")
                    nc.vector.tensor_tensor(t3[:], o_blk[0:96, :], cosq, ALU.mult)
                    nc.vector.tensor_tensor(t4[:], e_blk[0:96, :], sinq, ALU.mult)
                    nc.vector.tensor_tensor(orr[:, sl], t3[:], t4[:], ALU.add)

                if qn == 3:
                    # K first: the half-1 kt repack gates phase 2
                    B2 = mm_block(2)
                    if hook: hook()
                    B3 = mm_block(3)
                    rope(B2, B3, erk, ork)
                    if hook: hook()
                    B0 = mm_block(0)
                    if hook: hook()
                    B1 = mm_block(1)
                    rope(B0, B1, erq, orq)
                    if hook: hook()
                else:
                    B0 = mm_block(0)
                    B1 = mm_block(1)
                    rope(B0, B1, erq, orq)
                    B2 = mm_block(2)
                    B3 = mm_block(3)
                    rope(B2, B3, erk, ork)
                B4 = mm_block(4)
                # pass dims: Qp spread over B0/B1/B2 remainders, Kp over B3/B4.
                # The last quarter's copies go to the (idle) ACT engine so the
                # PSUM banks free up fast for phase 2.
                nc.scalar.copy(prq[0:32, sl], B0[96:128, :])
                nc.scalar.copy(prq[32:64, sl], B1[96:128, :])
                nc.scalar.copy(prq[64:96, sl], B2[96:128, :])
                nc.scalar.copy(prk[0:32, sl], B3[96:128, :])
                nc.scalar.copy(prk[32:64, sl], B4[0:32, :])
                nc.scalar.copy(prk[64:96, sl], B4[32:64, :])

            def emit_repack(hn, halves, part="both", quarter=None):
                """DMA the rotated halves into per-head [72, N] q/k tiles.

                Per-head dim order: [0:24] even-rotated, [24:48] odd-rotated,
                [48:72] pass -- same permutation for q and k, so scores are
                unchanged. Issues alternate between the two HWDGE engines.
                """
                erq, orq, prq, erk, ork, prk = halves[hn]
                if quarter is None:
                    cs, sz = slice(0, HS), HS
                else:
                    cs, sz = slice(quarter * QS, (quarter + 1) * QS), QS
                hs0 = hn * HS + (0 if quarter is None else quarter * QS)
                qdma = dma if hn == 0 else nc.gpsimd.dma_start
                if part in ("both", "kt"):
                    for h in range(HPG):
                        d0 = h * N + hs0
                        r = slice(24 * h, 24 * h + 24)
                        if hn == 1:
                            # ACT.SEQ is running partA exps when these issue;
                            # keep the critical kt repack off it entirely
                            eng = nc.gpsimd.dma_start if h % 2 else nc.sync.dma_start
                        else:
                            eng = dma
                        eng(kt16[0:24, d0:d0 + sz], erk[r, cs])
                        eng(kt16[24:48, d0:d0 + sz], ork[r, cs])
                        eng(kt16[48:72, d0:d0 + sz], prk[r, cs])
                if part in ("both", "qt"):
                    for h in range(HPG):
                        d0 = h * N + hs0
                        r = slice(24 * h, 24 * h + 24)
                        qdma(qt16[0:24, d0:d0 + sz], erq[r, cs])
                        qdma(qt16[24:48, d0:d0 + sz], orq[r, cs])
                        qdma(qt16[48:72, d0:d0 + sz], prq[r, cs])

            def emit_v_tt(qn, tt, ps_pool, ks=range(CK), box=None):
                """V for all 4 heads, one 128-token tile, x-stationary.
                ks selects the contraction slice so fillers can split the
                accumulation into small units (box carries the psum tile)."""
                ts0 = qn * QS
                if box is None:
                    box = {}
                if "vp" not in box:
                    box["vp"] = ps_pool.tile([128, QS], F32, tag="op", bufs=2,
                                             name=f"vps{qn}_{tt}")
                vp = box["vp"]
                for k in ks:
                    nc.tensor.matmul(
                        vp[:, 0:HPG * HD],
                        x16[k][:, ts0 + tt * 128:ts0 + (tt + 1) * 128],
                        wqk_t[k][:, 576:576 + HPG * HD],
                        start=(k == 0), stop=(k == CK - 1),
                    )
                if ks[-1] == CK - 1:
                    cp = nc.scalar.copy if qn < 2 else nc.vector.tensor_copy
                    cp(
                        v16[qn * 4 + tt][:, :, 0:HD],
                        vp[:, 0:HPG * HD].rearrange("p (h d) -> p h d", h=HPG),
                    )

            def emit_proj(ct, jq, ps_pool, sb_pool):
                op = ps_pool.tile([128, QS], F32, tag="op", bufs=2,
                                  name=f"op{ct}_{jq}")
                for i in range(HPG):
                    hd2 = HD + 1 if i == 3 else HD
                    nc.tensor.matmul(
                        op[:], wp4[0:hd2, i, ct * 128:(ct + 1) * 128],
                        o16[i][:, jq * QS:(jq + 1) * QS],
                        start=(i == 0), stop=(i == HPG - 1),
                    )
                osb = sb_pool.tile([128, QS], F32, tag="osb", bufs=3,
                                   name=f"osb{ct}_{jq}")
                # copies alternate DVE/Pool; out-DMA issues from SP (the ACT
                # sequencer is saturated with exps in phase 2)
                if ct % 2 == 0:
                    nc.gpsimd.tensor_copy(osb[:], op[:])
                else:
                    nc.vector.tensor_copy(osb[:], op[:])
                nc.sync.dma_start(
                    outT[ct * 128:(ct + 1) * 128, jq * QS:(jq + 1) * QS], osb[:]
                )

            # ================= phase 1: QKV + RoPE + repack =================
            _s2cm = tc.tile_pool(name="s2", bufs=1)
            s2 = _s2cm.__enter__()
            early_es = {}

            def emit_partA_kp(h, kp):
                """Scores+exp for one kp of chunk (h, jq=0), emitted inside
                phase 1 once the half-0 repack is in flight. Uses two [128,QS]
                PSUM tiles from the shared 'op' tag and f512 exps so no extra
                banks are needed."""
                hb = h * N
                if True:
                    sts = []
                    for i in range(2):
                        kt = 2 * kp + i
                        stx = ps1.tile([128, QS], F32, tag="op", bufs=2,
                                       name=f"stE{h}_{kp}_{i}")
                        nc.tensor.matmul(
                            stx[:],
                            kt16[:, hb + kt * 128:hb + (kt + 1) * 128],
                            qt16[:, hb:hb + QS],
                            start=True, stop=True,
                        )
                        sts.append(stx)
                    e = s2.tile([128, 2 * QS], F16, tag="e", bufs=23,
                                name=f"eE{h}_{kp}")
                    for i in range(2):
                        nc.scalar.activation(e[:, i * QS:(i + 1) * QS],
                                             sts[i][:], AF.Exp, scale=SCALE)
                    early_es.setdefault(h, []).append(e)

            def emit_partA(h, ps_pool, between=None):
                for kp in range(4):
                    if between is not None:
                        between()
                    emit_partA_kp(h, kp)

            with (
                tc.tile_pool(name="s1", bufs=1) as s1,
                tc.tile_pool(name="ps1", bufs=1, space="PSUM") as ps1,
            ):
                halves = [
                    tuple(
                        s1.tile([96, HS], F16, tag=f"{nm}", bufs=1,
                                name=f"{nm}_{hn}")
                        for nm in ("erq", "orq", "prq", "erk", "ork", "prk")
                    )
                    for hn in range(2)
                ]
                kp_ctr = [0]

                def hook():
                    if kp_ctr[0] < 4:
                        emit_partA_kp(0, kp_ctr[0])
                        kp_ctr[0] += 1

                for qn in range(4):
                    emit_qkrope(ps1, s1, qn, halves,
                                hook=hook if qn == 3 else None)
                    if qn == 1:
                        for tt in range(4):
                            emit_v_tt(0, tt, ps1)
                        emit_repack(0, halves)
                    if qn == 2:
                        for tt in range(4):
                            emit_v_tt(1, tt, ps1)
                        emit_repack(1, halves, part="kt", quarter=0)
                    if qn == 3:
                        emit_repack(1, halves, part="kt", quarter=1)
                        emit_partA(1, ps1)
                        emit_repack(1, halves, part="qt")

            # ================= phase 2: attention + projection ===============
            with tc.tile_pool(name="ps2", bufs=1, space="PSUM") as ps2:
                fillers = deque()
                for qn in (2, 3):
                    for tt in range(4):
                        vbox = {}
                        for ks in (range(0, 3), range(3, 6), range(6, CK)):
                            fillers.append((288 * len(ks),
                                            lambda qn=qn, tt=tt, ks=ks, vbox=vbox:
                                            emit_v_tt(qn, tt, ps2, ks, vbox)))

                def mk_proj_filler(ct, jq):
                    # two units: heads 0-1, then heads 2-3 + copy + store
                    pbox = {}

                    def a():
                        pbox["op"] = ps2.tile([128, QS], F32, tag="op", bufs=2,
                                              name=f"op{ct}_{jq}")
                        for i in (0, 1):
                            nc.tensor.matmul(
                                pbox["op"][:],
                                wp4[0:HD, i, ct * 128:(ct + 1) * 128],
                                o16[i][:, jq * QS:(jq + 1) * QS],
                                start=(i == 0), stop=False,
                            )
                        return 1024

                    def b():
                        op = pbox["op"]
                        for i in (2, 3):
                            hd2 = HD + 1 if i == 3 else HD
                            nc.tensor.matmul(
                                op[:], wp4[0:hd2, i, ct * 128:(ct + 1) * 128],
                                o16[i][:, jq * QS:(jq + 1) * QS],
                                start=False, stop=(i == 3),
                            )
                        osb = s2.tile([128, QS], F32, tag="osb", bufs=3,
                                      name=f"osb{ct}_{jq}")
                        nc.vector.tensor_copy(osb[:], op[:])
                        nc.sync.dma_start(
                            outT[ct * 128:(ct + 1) * 128,
                                 jq * QS:(jq + 1) * QS], osb[:]
                        )
                        return 1664
                    return [(1024, a), (1664, b)]

                def mk_proj_ab(ct, jq):
                    pbox = {}

                    def a():
                        op = ps2.tile([128, QS], F32, tag="op", bufs=2,
                                      name=f"opA{ct}_{jq}")
                        for i in (0, 1):
                            nc.tensor.matmul(
                                op[:], wp4[0:HD, i, ct * 128:(ct + 1) * 128],
                                o16[i][:, jq * QS:(jq + 1) * QS],
                                start=(i == 0), stop=(i == 1),
                            )
                        park = s2.tile([128, QS], F16, tag="park", bufs=9,
                                       name=f"park{ct}")
                        nc.vector.tensor_copy(park[:], op[:])
                        pbox["park"] = park
                        return 1024

                    def b():
                        op = ps2.tile([128, QS], F32, tag="op", bufs=2,
                                      name=f"opB{ct}_{jq}")
                        for i in (2, 3):
                            hd2 = HD + 1 if i == 3 else HD
                            nc.tensor.matmul(
                                op[:], wp4[0:hd2, i, ct * 128:(ct + 1) * 128],
                                o16[i][:, jq * QS:(jq + 1) * QS],
                                start=(i == 2), stop=(i == 3),
                            )
                        osb = s2.tile([128, QS], F32, tag="osb", bufs=3,
                                      name=f"osb{ct}_{jq}")
                        nc.vector.tensor_tensor(osb[:], op[:],
                                                pbox["park"][:], ALU.add)
                        nc.sync.dma_start(
                            outT[ct * 128:(ct + 1) * 128,
                                 jq * QS:(jq + 1) * QS], osb[:]
                        )
                        return 1664
                    return (1024, a), (1664, b)

                def drain(budget):
                    while fillers and budget > 0:
                        cost, fn = fillers.popleft()
                        fn()
                        budget -= cost

                def emit_scores(h, jq, first=False):
                    hb = h * N
                    es = []
                    kps = range(8)
                    if jq == 0 and h in early_es:
                        es = list(early_es[h])
                        kps = range(4, 8)
                    for kp in kps:
                        st = ps2.tile([128, 2 * QS], F32, tag="st", bufs=2,
                                      name=f"st{h}_{jq}_{kp}")
                        for i in range(2):
                            kt = 2 * kp + i
                            nc.tensor.matmul(
                                st[:, i * QS:(i + 1) * QS],
                                kt16[:, hb + kt * 128:hb + (kt + 1) * 128],
                                qt16[:, hb + jq * QS:hb + (jq + 1) * QS],
                                start=True, stop=True,
                            )
                        e = s2.tile([128, 2 * QS], F16, tag="e", bufs=23,
                                    name=f"e{h}_{jq}_{kp}")
                        nc.scalar.activation(e[:], st[:], AF.Exp, scale=SCALE)
                        es.append(e)
                        if kps[0] == 4:
                            drain(2100)
                        elif jq == NQ - 1:
                            drain(0 if kp < 2 else 2300)
                        else:
                            drain(0 if kp < 2 else 2100)
                    return es

                av_ready = deque()

                def tail_units(h, jq, es):
                    """AV + normalize + transpose for a finished chunk, as filler units."""
                    units = []
                    box = {}

                    def mk_av(kp):
                        def f():
                            if kp == 0:
                                if av_ready:
                                    box["av"] = av_ready.popleft()
                                else:
                                    box["av"] = ps2.tile(
                                        [128, NQ, HD + 1], F32, tag="av",
                                        bufs=1, name=f"av{h}_{jq}")
                                    # 4 accumulation regions share one PSUM
                                    # bank; a start=True zeroes the whole
                                    # bank, so zero once, accumulate with
                                    # start=False
                                    nc.vector.memset(box["av"][:], 0.0)
                            av = box["av"]
                            for i in range(2):
                                kt = 2 * kp + i
                                for qi in range(NQ):
                                    nc.tensor.matmul(
                                        av[:, qi, :],
                                        es[kp][:, i * QS + qi * 128:
                                               i * QS + (qi + 1) * 128],
                                        v16[kt][:, h, :],
                                        start=False, stop=(kt == KTILES - 1),
                                        skip_group_check=True,
                                    )
                            return 584
                        return f

                    for kp in range(8):
                        units.append((584, mk_av(kp)))

                    def f_fin():
                        av = box["av"]
                        rec = s2.tile([128, NQ], F32, tag="rec", bufs=2,
                                      name=f"rec{h}_{jq}")
                        nc.vector.reciprocal(rec[:], av[:, :, HD])
                        o_n = s2.tile([128, NQ, HD], F16, tag="on", bufs=2,
                                      name=f"on{h}_{jq}")
                        for qi in range(NQ):
                            nc.vector.tensor_scalar_mul(
                                o_n[:, qi, :], av[:, qi, 0:HD], rec[:, qi:qi + 1]
                            )
                        nxt = ps2.tile([128, NQ, HD + 1], F32, tag="av",
                                       bufs=1, name=f"avn{h}_{jq}")
                        nc.vector.memset(nxt[:], 0.0)
                        av_ready.append(nxt)
                        pt = ps2.tile([HD, NQ, 128], F16, tag="pt", bufs=1,
                                      name=f"pt{h}_{jq}")
                        for qi in range(NQ):
                            nc.tensor.transpose(pt[:, qi, :], o_n[:, qi, :],
                                                ident[:])
                        nc.vector.tensor_copy(
                            o16[h][0:HD, jq * QS:(jq + 1) * QS], pt[:]
                        )
                        return 512
                    units.append((512, f_fin))
                    return units

                ab_units = [mk_proj_ab(ct, NQ - 1) for ct in range(CK)]
                prev = None
                for jq in range(NQ):
                    for h in range(HPG):
                        if prev is not None:
                            fillers.extend(tail_units(*prev))
                            if prev[0] == 3 and prev[1] < NQ - 1:
                                for ct in range(CK):
                                    fillers.extend(mk_proj_filler(ct, prev[1]))
                            if prev[0] == 1 and prev[1] == NQ - 1:
                                fillers.extend(u[0] for u in ab_units)
                        es = emit_scores(h, jq, first=(jq == 0 and h == 0))
                        prev = (h, jq, es)
                fillers.extend(tail_units(*prev))
                fillers.extend(u[1] for u in ab_units)
                drain(10 ** 9)
            _s2cm.__exit__(None, None, None)

    bass_rust.generate_event_semaphores(nc)
    return nc


_NC = None


def _get_nc():
    global _NC
    if _NC is None:
        _NC = build_nc()
    return _NC


def kernel(x, Wqkv, Wproj, bproj, T, H, W):
    x = np.asarray(x, dtype=np.float32)
    Wqkv = np.asarray(Wqkv, dtype=np.float32)
    Wproj = np.asarray(Wproj, dtype=np.float32)
    bproj = np.asarray(bproj, dtype=np.float32)
    assert x.shape == (B, N, C) and Wqkv.shape == (C, 3 * C)
    assert (int(T), int(H), int(W)) == (GT, GH, GW)

    cos96, sin96 = _cos_sin_96()
    nc = _get_nc()

    in_maps = []
    for core in range(NCORES):
        b, g = divmod(core, HPG)
        heads = [HPG * g + i for i in range(HPG)]
        qe = [h * HD + 2 * j for h in heads for j in range(24)]
        qo = [h * HD + 2 * j + 1 for h in heads for j in range(24)]
        qp = [h * HD + 48 + j for h in heads for j in range(24)]
        ke = [C + i for i in qe]
        ko = [C + i for i in qo]
        kp = [C + i for i in qp]
        vcols = [2 * C + h * HD + d for h in heads for d in range(HD)]
        cols = (qe + qp[0:32] + qo + qp[32:64] + ke + qp[64:96]
                + ko + kp[0:32] + kp[32:96] + vcols)
        wqk_c = Wqkv[:, cols].astype(np.float16)
        bias_row = bproj if g == 0 else np.zeros_like(bproj)
        # wp packed [73, HPG*C]: head h columns [h*C:(h+1)*C], row 72 = bias
        # (only meaningful for h==3, whose o16 carries the ones row)
        wp_c = np.zeros((HD + 1, HPG * C), dtype=np.float32)
        for i, h in enumerate(heads):
            wp_c[0:HD, i * C:(i + 1) * C] = Wproj[h * HD:(h + 1) * HD, :]
        wp_c[HD, 3 * C:4 * C] = bias_row
        in_maps.append({
            "xT": np.ascontiguousarray(x[b].T).astype(np.float16),
            "wqk": np.ascontiguousarray(wqk_c),
            "wp": wp_c.astype(np.float16),
            "cosd": cos96.astype(np.float16),
            "sind": sin96.astype(np.float16),
        })

    res = run_bass_kernel_spmd(nc, in_maps, core_ids=list(range(NCORES)))
    out = np.zeros((B, N, C), dtype=np.float32)
    for core in range(NCORES):
        b = core // HPG
        out[b] += res.results[core]["outT"].T
    return out
